# revision 1
# baseline (speedup 1.0000x reference)
"""Trainium2 Bass kernel for AngelLoss (center loss + angular loss).

loss = 0.5*sum((feat - centers[y])^2)/B
     + sum_offdiag((c_i.c_j/(|c_i||c_j|) - ct)^2) / (0.5*C*(C-1))

Sharding (8 NeuronCores):
  - batch term: feat/y sharded along batch (8192 rows/core); each core
    dma_gathers its centers rows (bf16 table) in 9 chunks, subtracts on
    DVE, and square-reduces on ScalarE into per-partition partials.
  - angular term: center rows sharded (125 rows/core); each core computes
    its 125x1000 slice of the normalized Gram matrix on TensorE.
  - per-core [1,16] partial sums are combined on the host.
"""

from contextlib import ExitStack

import ml_dtypes
import numpy as np

import concourse.bass as bass
import concourse.tile as tile
from concourse import bacc, mybir
from concourse.bass import ds, ts
from concourse.bass_utils import run_bass_kernel_spmd

N_CORES = 8
B, C, D = 65536, 1000, 512
BS = B // N_CORES  # 8192 rows per core
CHUNK_SIZES = [1024] * 7 + [512, 512]  # small tail chunks shorten the serial tail
CHUNKS = len(CHUNK_SIZES)
MAXSLOTS = max(CHUNK_SIZES) // 128
CS = C // N_CORES  # 125 gram rows per core

# ct = 2*radius(C-1)^2 - 1 from the reference, evaluated in f64, cast f32.
CT = float(np.float32(-0.0010010010010047532))

_F32 = mybir.dt.float32
_BF16 = mybir.dt.bfloat16
_I16 = mybir.dt.int16

_NC_CACHE = {}


def _build_body(ctx, tc, feat, cslice, idx16, identity, cbf, out):
    nc = tc.nc
    AF = mybir.ActivationFunctionType

    const = ctx.enter_context(tc.tile_pool(name="const", bufs=1))
    pnrm = ctx.enter_context(tc.tile_pool(name="nrm", bufs=3))
    pfeat = ctx.enter_context(tc.tile_pool(name="feat", bufs=3))
    pgath = ctx.enter_context(tc.tile_pool(name="gath", bufs=4))
    pscr = ctx.enter_context(tc.tile_pool(name="scr", bufs=2))
    ptp = ctx.enter_context(tc.tile_pool(name="tp", bufs=2, space="PSUM"))
    pgram = ctx.enter_context(tc.tile_pool(name="gram", bufs=2, space="PSUM"))
    pfin = ctx.enter_context(tc.tile_pool(name="fin", bufs=1, space="PSUM"))

    idxt = const.tile([128, BS // 16], _I16)
    nc.sync.dma_start(idxt[:], idx16[:, :])

    # Warm up the SWDGE gather path (library load + Q7 code fetch) while the
    # table loads. Keep ALL other work off gpsimd so the library switch is
    # the very first thing the engine does.
    warm = const.tile([128, 1, D], _BF16)
    nc.gpsimd.dma_gather(warm[:], cbf[:, :], idxt[:, 0:8], 128, 128, D)

    # identity comes in as an input; constants built on DVE (NOT gpsimd,
    # to keep the mlp-library switch at the head of the gpsimd stream)
    ident = const.tile([128, 128], _BF16)
    nc.sync.dma_start(ident[:], identity[:, :])
    ones = const.tile([128, 1], _F32)
    nc.vector.memset(ones[:], 1.0)
    # staging[:, 0:8]: per-chunk center-loss partials; [:, 8:10]: angular
    # halves; [:, 15]: warmup-gather consumer (ignored by the host).
    staging = const.tile([128, 16], _F32)
    nc.vector.memset(staging[:], 0.0)
    ctbias = const.tile([128, 1], _F32)
    nc.vector.memset(ctbias[:], -CT)

    # --- one-shot centers load -> bf16 table store (cbf) ---
    # partition p holds rows p*8 .. p*8+7 (contiguous 16 KiB per partition)
    ct_all = const.tile([125, 8, D], _BF16)
    nc.scalar.dma_start(ct_all[:], cbf.rearrange("(p s) d -> p s d", p=125))
    tslice = const.tile([128, D], _F32)
    nc.sync.dma_start(tslice[:CS, :], cslice[:, :])

    # --- angular term (fills the window while gathers/feat stream) ---
    # normalize the [125, 8, D] resident center rows; per-(p,s) row norms via
    # ScalarE square + fused free-axis accumulate, one slot column at a time
    nsq = const.tile([125, 8], _F32)
    for s in range(8):
        ttr = pnrm.tile([125, D], _F32, tag="ttr")
        nc.scalar.activation(
            ttr[:], ct_all[:, s, :], AF.Square, accum_out=nsq[:, s : s + 1]
        )
    sd = pnrm.tile([125, 8], _F32, tag="sd")
    nc.scalar.activation(sd[:], nsq[:], AF.Sqrt)
    inv = pnrm.tile([125, 8], _F32, tag="inv")
    nc.vector.reciprocal(inv[:], sd[:])
    cn_all = const.tile([125, 8, D], _BF16)
    nc.vector.tensor_tensor(
        out=cn_all[:],
        in0=ct_all[:],
        in1=inv[:, :].to_broadcast([125, 8, D]),
        op=mybir.AluOpType.mult,
    )
    # normalize the slice rows ([125, D], one row per partition)
    sq2 = pnrm.tile([128, D], _F32, tag="sq2")
    nsq2 = pnrm.tile([128, 1], _F32, tag="nsq2")
    nc.scalar.activation(sq2[:CS, :], tslice[:CS, :], AF.Square, accum_out=nsq2[:CS, :])
    sd2 = pnrm.tile([128, 1], _F32, tag="sd2")
    nc.scalar.activation(sd2[:CS, :], nsq2[:CS, :], AF.Sqrt)
    inv2 = pnrm.tile([128, 1], _F32, tag="inv2")
    nc.vector.reciprocal(inv2[:CS, :], sd2[:CS, :])
    cns = const.tile([128, D], _BF16)
    nc.scalar.activation(cns[:CS, :], tslice[:CS, :], AF.Copy, scale=inv2[:CS, :])

    # transposed normalized centers; class (q, s) = row q*8+s in column (q, s)
    _hp = tc.high_priority()
    _hp.__enter__()
    cnT = const.tile([128, 4, 125, 8], _BF16)
    for s in range(8):
        for ki in range(4):
            pt = ptp.tile([128, 128], _BF16, tag="tp")
            nc.tensor.transpose(pt[:, :125], cn_all[:, s, ts(ki, 128)], ident[:125, :125])
            nc.vector.tensor_copy(cnT[:, ki, :, s : s + 1], pt[:, :125])
    cnTs = const.tile([128, 4, CS], _BF16)
    for ki in range(4):
        pt = ptp.tile([128, 128], _BF16, tag="tp")
        nc.tensor.transpose(pt[:, :CS], cns[:CS, ts(ki, 128)], ident[:CS, :CS])
        nc.vector.tensor_copy(cnTs[:, ki, :], pt[:, :CS])

    # gram slice: [125 rows, 1000 classes] in two column halves
    HALVES = [(0, 63), (63, 62)]
    grams = []
    for q0, qn in HALVES:
        pg = pgram.tile([CS, 63 * 8], _F32, tag="gram")
        for ki in range(4):
            nc.tensor.matmul(
                pg[:, : qn * 8],
                cnTs[:, ki, :],
                cnT[:, ki, ds(q0, qn), :],
                start=(ki == 0),
                stop=(ki == 3),
            )
        grams.append((pg, qn))
    _hp.__exit__(None, None, None)

    # --- center loss loop ---
    row0 = 0
    for c, csz in enumerate(CHUNK_SIZES):
        slots = csz // 128
        gt = pgath.tile([128, MAXSLOTS, D], _BF16, tag="gt")
        nc.gpsimd.dma_gather(
            gt[:, :slots, :],
            cbf[:, :],
            idxt[:, ds(row0 // 16, csz // 16)],
            csz,
            csz,
            D,
        )
        ft = pfeat.tile([128, MAXSLOTS, D], _F32, tag="ft")
        # partition p holds rows [row0 + p*slots, ...+slots): contiguous
        nc.sync.dma_start(
            ft[:, :slots, :],
            feat[ds(row0, csz), :].rearrange("(p s) d -> p s d", p=128),
        )
        st = pscr.tile([128, MAXSLOTS, D], _F32, tag="st")
        nc.vector.tensor_tensor(
            out=st[:, :slots, :],
            in0=ft[:, :slots, :],
            in1=gt[:, :slots, :],
            op=mybir.AluOpType.subtract,
        )
        nc.scalar.activation(
            ft[:, :slots, :],
            st[:, :slots, :],
            AF.Square,
            accum_out=staging[:, c : c + 1],
        )
        row0 += csz

    # --- angular squares + final partition collapse ---
    for h, (pg, qn) in enumerate(grams):
        gs = pnrm.tile([CS, 63 * 8], _F32, tag="gscr")
        nc.scalar.activation(
            gs[:, : qn * 8],
            pg[:, : qn * 8],
            AF.Square,
            bias=ctbias[:CS, :],
            accum_out=staging[:CS, 10 + h : 11 + h],
        )
    pf = pfin.tile([1, 16], _F32, tag="fin")
    nc.tensor.matmul(pf[:], ones[:], staging[:], start=True, stop=True)
    osb = const.tile([1, 16], _F32)
    nc.vector.tensor_copy(osb[:], pf[:])
    nc.vector.tensor_copy(osb[0:1, 15:16], warm[0:1, 0, 0:1])
    nc.sync.dma_start(out[:, :], osb[:, :])


def build():
    if "nc" in _NC_CACHE:
        return _NC_CACHE["nc"]
    nc = bacc.Bacc(
        "TRN2",
        target_bir_lowering=False,
        debug=False,
        enable_asserts=False,
        num_devices=N_CORES,
    )
    feat = nc.dram_tensor("feat", [BS, D], _F32, kind="ExternalInput").ap()
    cslice = nc.dram_tensor("cslice", [CS, D], _F32, kind="ExternalInput").ap()
    idx16 = nc.dram_tensor("idx16", [128, BS // 16], _I16, kind="ExternalInput").ap()
    identity = nc.dram_tensor("identity", [128, 128], _BF16, kind="ExternalInput").ap()
    cbf = nc.dram_tensor("ctab", [C, D], _BF16, kind="ExternalInput").ap()
    out = nc.dram_tensor("out", [1, 16], _F32, kind="ExternalOutput").ap()
    with tile.TileContext(nc) as tc, ExitStack() as ctx:
        _build_body(ctx, tc, feat, cslice, idx16, identity, cbf, out)
    nc.compile()
    _NC_CACHE["nc"] = nc
    return nc


def make_in_maps(y, feat, centers):
    feat = np.ascontiguousarray(feat, dtype=np.float32)
    centers = np.ascontiguousarray(centers, dtype=np.float32)
    y = np.asarray(y)
    ctab = centers.astype(ml_dtypes.bfloat16)
    in_maps = []
    for i in range(N_CORES):
        ys = y[i * BS : (i + 1) * BS].astype(np.int16)
        # gather position j in a chunk pairs with feat row row0 + (j%128)*slots + j//128
        parts = []
        row0 = 0
        for csz in CHUNK_SIZES:
            slots = csz // 128
            j = np.arange(csz)
            parts.append(ys[row0 + (j % 128) * slots + j // 128])
            row0 += csz
        yp = np.concatenate(parts)
        # [16, BS/16] stripes (position j at [j%16, j//16]), replicated into all
        # eight 16-partition groups (each SWDGE Q7 core reads its own stripe).
        idx = np.tile(yp.reshape(BS // 16, 16).T, (8, 1))
        in_maps.append(
            {
                "feat": np.ascontiguousarray(feat[i * BS : (i + 1) * BS]),
                "cslice": np.ascontiguousarray(centers[i * CS : (i + 1) * CS]),
                "idx16": idx,
                "identity": np.eye(128, dtype=ml_dtypes.bfloat16),
                "ctab": ctab,
            }
        )
    return in_maps


def combine(outs):
    """outs: list of 8 [1,16] f32 arrays -> scalar loss (np.float32)."""
    cen = 0.0
    ang = 0.0
    for o in outs:
        o = np.asarray(o, dtype=np.float64)
        cen += o[0, 0:9].sum()
        ang += o[0, 10:12].sum()
    ang -= C * (1.0 - CT) ** 2  # remove the diagonal (sim_ii == 1) terms
    loss = 0.5 * cen / B + ang / (0.5 * C * (C - 1))
    return np.float32(loss)


def kernel(y, feat, centers):
    nc = build()
    in_maps = make_in_maps(y, feat, centers)
    res = run_bass_kernel_spmd(nc, in_maps, core_ids=list(range(N_CORES)))
    return combine([res.results[i]["out"] for i in range(N_CORES)])



# revision 4
# speedup vs baseline: 1.4690x; 1.4690x over previous
"""Trainium2 Bass kernel for AngelLoss (center loss + angular loss).

loss = 0.5*sum((feat - centers[y])^2)/B
     + sum_offdiag((c_i.c_j/(|c_i||c_j|) - ct)^2) / (0.5*C*(C-1))

Sharding (8 NeuronCores, data-parallel over batch):
  - center term, gather-free:  sum||f||^2 - 2*sum_c c_c.S_c + sum_c n_c||c_c||^2
    where S_c = sum of feat rows with label c.  Host buckets each core's
    8192 rows into 8 class-banks (125 classes each, greedy-balanced), pads
    each bank's rows to 9x128, and ships a per-subtile onehot.  S for one
    bank accumulates in one PSUM tile via 9 onehot^T @ feat matmuls, then
    drains with a fused DVE multiply-reduce against the resident centers.
    sum||f||^2 streams on ScalarE (square + free-axis accumulate).
  - angular term: center rows sharded (125 rows/core); each core computes
    its 125x1000 slice of the normalized Gram matrix on TensorE.
  - per-core [1,32] partial sums are combined on the host.
"""

from contextlib import ExitStack

import ml_dtypes
import numpy as np

import concourse.bass as bass
import concourse.tile as tile
from concourse import bacc, mybir
from concourse.bass import ds, ts
from concourse.bass_utils import run_bass_kernel_spmd

N_CORES = 8
B, C, D = 65536, 1000, 512
BS = B // N_CORES  # 8192 rows per core
NB = 8  # class banks
CPB = C // NB  # 125 classes per bank
SUB = 9  # 128-row subtiles per bank group (1152 slots >= ~1024+slack rows)
GROUP = 128 * SUB  # 1152
PR = NB * GROUP  # 9216 padded rows per core
CS = C // N_CORES  # 125 gram rows per core

# ct = 2*radius(C-1)^2 - 1 from the reference, evaluated in f64, cast f32.
CT = float(np.float32(-0.0010010010010047532))

_F32 = mybir.dt.float32
_BF16 = mybir.dt.bfloat16

_NC_CACHE = {}


def _build_body(ctx, tc, feat, cs2, oh, identity, cbf, out):
    nc = tc.nc
    AF = mybir.ActivationFunctionType

    const = ctx.enter_context(tc.tile_pool(name="const", bufs=1))
    pnrm = ctx.enter_context(tc.tile_pool(name="nrm", bufs=3))
    pfeat = ctx.enter_context(tc.tile_pool(name="feat", bufs=3))
    psq = ctx.enter_context(tc.tile_pool(name="sq", bufs=2))
    pdscr = ctx.enter_context(tc.tile_pool(name="dscr", bufs=2))
    ptp = ctx.enter_context(tc.tile_pool(name="tp", bufs=2, space="PSUM"))
    pgram = ctx.enter_context(tc.tile_pool(name="gram", bufs=2, space="PSUM"))
    pS = ctx.enter_context(tc.tile_pool(name="S", bufs=2, space="PSUM"))
    pfin = ctx.enter_context(tc.tile_pool(name="fin", bufs=1, space="PSUM"))

    ident = const.tile([128, 128], _BF16)
    nc.sync.dma_start(ident[:], identity[:, :])
    ones = const.tile([128, 1], _F32)
    nc.vector.memset(ones[:], 1.0)
    # staging cols: 0-7 sum(f^2) per group; 8 counts.|c|^2; 10-17 -2*cross
    # per bank; 18-19 angular halves.
    staging = const.tile([128, 32], _F32)
    nc.vector.memset(staging[:], 0.0)
    ctbias = const.tile([128, 1], _F32)
    nc.vector.memset(ctbias[:], -CT)

    # onehot tiles, one [128, SUB, 128] load per bank group
    oht = const.tile([128, NB * SUB, 128], _BF16)
    for g in range(NB):
        nc.scalar.dma_start(
            oht[:, ds(g * SUB, SUB), :], oh[:, ds(g * SUB, SUB), :]
        )

    # centers table, bank-major: (p, j) holds class bankclasses[j][p]
    ct_all = const.tile([CPB, NB, D], _BF16)
    nc.scalar.dma_start(ct_all[:], cbf.rearrange("(p s) d -> p s d", p=CPB))
    # gram row slice (f32) + per-class counts packed in cols 512:520
    tslice = const.tile([128, 520], _F32)
    nc.sync.dma_start(tslice[:CS, :], cs2[:, :])

    # --- angular term (fills the window while feat streams) ---
    nsq = const.tile([CPB, NB], _F32)
    for s in range(NB):
        ttr = pnrm.tile([CPB, D], _F32, tag="ttr")
        nc.scalar.activation(
            ttr[:], ct_all[:, s, :], AF.Square, accum_out=nsq[:, s : s + 1]
        )
    sd = pnrm.tile([CPB, NB], _F32, tag="sd")
    nc.scalar.activation(sd[:], nsq[:], AF.Sqrt)
    inv = pnrm.tile([CPB, NB], _F32, tag="inv")
    nc.vector.reciprocal(inv[:], sd[:])
    cn_all = const.tile([CPB, NB, D], _BF16)
    nc.vector.tensor_tensor(
        out=cn_all[:],
        in0=ct_all[:],
        in1=inv[:, :].to_broadcast([CPB, NB, D]),
        op=mybir.AluOpType.mult,
    )
    # normalize the slice rows ([125, D], one row per partition)
    sq2 = pnrm.tile([128, D], _F32, tag="sq2")
    nsq2 = pnrm.tile([128, 1], _F32, tag="nsq2")
    nc.scalar.activation(
        sq2[:CS, :], tslice[:CS, :D], AF.Square, accum_out=nsq2[:CS, :]
    )
    sd2 = pnrm.tile([128, 1], _F32, tag="sd2")
    nc.scalar.activation(sd2[:CS, :], nsq2[:CS, :], AF.Sqrt)
    inv2 = pnrm.tile([128, 1], _F32, tag="inv2")
    nc.vector.reciprocal(inv2[:CS, :], sd2[:CS, :])
    cns = const.tile([128, D], _BF16)
    nc.scalar.activation(cns[:CS, :], tslice[:CS, :D], AF.Copy, scale=inv2[:CS, :])

    # counts . |c|^2 term
    cscr = pnrm.tile([CPB, NB], _F32, tag="cscr")
    nc.vector.tensor_tensor(
        out=cscr[:],
        in0=nsq[:],
        in1=tslice[:CS, 512:520],
        op=mybir.AluOpType.mult,
    )
    cscr2 = pnrm.tile([CPB, NB], _F32, tag="cscr2")
    nc.scalar.activation(
        cscr2[:], cscr[:], AF.Copy, accum_out=staging[:CS, 8:9]
    )

    # transposed normalized centers; class (q, s) = table row q*8+s -> col (q, s)
    _hp = tc.high_priority()
    _hp.__enter__()
    cnT = const.tile([128, 4, CPB, NB], _BF16)
    for s in range(NB):
        for ki in range(4):
            pt = ptp.tile([128, 128], _BF16, tag="tp")
            nc.tensor.transpose(pt[:, :CPB], cn_all[:, s, ts(ki, 128)], ident[:CPB, :CPB])
            nc.vector.tensor_copy(cnT[:, ki, :, s : s + 1], pt[:, :CPB])
    cnTs = const.tile([128, 4, CS], _BF16)
    for ki in range(4):
        pt = ptp.tile([128, 128], _BF16, tag="tp")
        nc.tensor.transpose(pt[:, :CS], cns[:CS, ts(ki, 128)], ident[:CS, :CS])
        nc.vector.tensor_copy(cnTs[:, ki, :], pt[:, :CS])

    # gram slice: [125 rows, 1000 classes] in two column halves
    HALVES = [(0, 63), (63, 62)]
    grams = []
    for q0, qn in HALVES:
        pg = pgram.tile([CS, 63 * NB], _F32, tag="gram")
        for ki in range(4):
            nc.tensor.matmul(
                pg[:, : qn * NB],
                cnTs[:, ki, :],
                cnT[:, ki, ds(q0, qn), :],
                start=(ki == 0),
                stop=(ki == 3),
            )
        grams.append((pg, qn))
    _hp.__exit__(None, None, None)

    # --- center-loss main loop: one PSUM scatter bank per group ---
    for g in range(NB):
        ft = pfeat.tile([128, SUB, D], _BF16, tag="ft")
        nc.sync.dma_start(
            ft[:], feat[ds(g * GROUP, GROUP), :].rearrange("(p s) d -> p s d", p=128)
        )
        sq = psq.tile([128, SUB, D], _BF16, tag="sq")
        nc.scalar.activation(sq[:], ft[:], AF.Square, accum_out=staging[:, g : g + 1])
        st = pS.tile([128, D], _F32, tag="S")
        for s in range(SUB):
            nc.tensor.matmul(
                st[:],
                oht[:, g * SUB + s, :],
                ft[:, s, :],
                start=(s == 0),
                stop=(s == SUB - 1),
            )
        dscr = pdscr.tile([CPB, D], _F32, tag="dscr")
        nc.vector.tensor_tensor(
            out=dscr[:],
            in0=st[:CPB, :],
            in1=ct_all[:, g, :],
            op=mybir.AluOpType.mult,
        )
        dscr2 = pdscr.tile([CPB, D], _BF16, tag="dscr2")
        nc.scalar.activation(
            dscr2[:],
            dscr[:],
            AF.Copy,
            scale=-2.0,
            accum_out=staging[:CPB, 10 + g : 11 + g],
        )

    # --- angular squares + final partition collapse ---
    for h, (pg, qn) in enumerate(grams):
        gs = pnrm.tile([CS, 63 * NB], _F32, tag="gscr")
        nc.scalar.activation(
            gs[:, : qn * NB],
            pg[:, : qn * NB],
            AF.Square,
            bias=ctbias[:CS, :],
            accum_out=staging[:CS, 18 + h : 19 + h],
        )
    pf = pfin.tile([1, 32], _F32, tag="fin")
    nc.tensor.matmul(pf[:], ones[:], staging[:], start=True, stop=True)
    osb = const.tile([1, 32], _F32)
    nc.vector.tensor_copy(osb[:], pf[:])
    nc.sync.dma_start(out[:, :], osb[:, :])


def build():
    if "nc" in _NC_CACHE:
        return _NC_CACHE["nc"]
    nc = bacc.Bacc(
        "TRN2",
        target_bir_lowering=False,
        debug=False,
        enable_asserts=False,
        num_devices=N_CORES,
    )
    feat = nc.dram_tensor("feat", [PR, D], _BF16, kind="ExternalInput").ap()
    cs2 = nc.dram_tensor("cs2", [CS, 520], _F32, kind="ExternalInput").ap()
    oh = nc.dram_tensor("oh", [128, NB * SUB, 128], _BF16, kind="ExternalInput").ap()
    identity = nc.dram_tensor("identity", [128, 128], _BF16, kind="ExternalInput").ap()
    cbf = nc.dram_tensor("ctab", [C, D], _BF16, kind="ExternalInput").ap()
    out = nc.dram_tensor("out", [1, 32], _F32, kind="ExternalOutput").ap()
    with tile.TileContext(nc) as tc, ExitStack() as ctx:
        _build_body(ctx, tc, feat, cs2, oh, identity, cbf, out)
    nc.compile()
    _NC_CACHE["nc"] = nc
    return nc


def _bank_assignment(y):
    """Greedy-balanced partition of the C classes into NB banks of CPB each.

    Returns (bankclasses [NB, CPB] int array, class->(bank, pos) maps).
    """
    counts = np.bincount(y, minlength=C)
    order = np.argsort(-counts, kind="stable")
    bank_tot = np.zeros(NB, dtype=np.int64)
    bank_n = np.zeros(NB, dtype=np.int64)
    bankclasses = np.zeros((NB, CPB), dtype=np.int64)
    cls_bank = np.zeros(C, dtype=np.int64)
    cls_pos = np.zeros(C, dtype=np.int64)
    for c in order:
        open_banks = np.flatnonzero(bank_n < CPB)
        j = open_banks[np.argmin(bank_tot[open_banks])]
        bankclasses[j, bank_n[j]] = c
        cls_bank[c] = j
        cls_pos[c] = bank_n[j]
        bank_n[j] += 1
        bank_tot[j] += counts[c]
    assert bank_tot.max() <= GROUP, f"bank overflow: {bank_tot.max()} > {GROUP}"
    return bankclasses, cls_bank, cls_pos, counts


def make_in_maps(y, feat, centers):
    feat = np.ascontiguousarray(feat, dtype=np.float32)
    centers = np.ascontiguousarray(centers, dtype=np.float32)
    y = np.asarray(y).astype(np.int64)
    ident = np.eye(128, dtype=ml_dtypes.bfloat16)
    in_maps = []
    for i in range(N_CORES):
        ys = y[i * BS : (i + 1) * BS]
        fs = feat[i * BS : (i + 1) * BS]
        bankclasses, cls_bank, cls_pos, counts = _bank_assignment(ys)

        # bank-major centers table: dram row r = class bankclasses[r % NB][r // NB]
        perm_tab = bankclasses[np.arange(C) % NB, np.arange(C) // NB]
        ctab = centers[perm_tab].astype(ml_dtypes.bfloat16)

        # bucket rows by bank; group g rows sit at slots [g*GROUP, g*GROUP+n_g)
        row_bank = cls_bank[ys]
        grp_order = np.argsort(row_bank, kind="stable")  # rows sorted by bank
        n_per = np.bincount(row_bank, minlength=NB)
        starts = np.zeros(NB + 1, dtype=np.int64)
        starts[1:] = np.cumsum(n_per)
        slot = np.full(PR, -1, dtype=np.int64)  # slot -> source row
        for g in range(NB):
            rows = grp_order[starts[g] : starts[g + 1]]
            slot[g * GROUP : g * GROUP + len(rows)] = rows

        featp = np.zeros((PR, D), dtype=ml_dtypes.bfloat16)
        valid = slot >= 0
        featp[valid] = fs[slot[valid]].astype(ml_dtypes.bfloat16)

        # onehot: matmul (g, s) covers slots g*GROUP + 9p + s for p in [0,128)
        oh = np.zeros((128, NB * SUB, 128), dtype=ml_dtypes.bfloat16)
        k = np.flatnonzero(valid)
        g_k = k // GROUP
        r_k = k % GROUP
        p_k = r_k // SUB
        s_k = r_k % SUB
        col_k = cls_pos[ys[slot[k]]]
        oh[p_k, g_k * SUB + s_k, col_k] = 1.0

        # per-(pos, bank) counts for the n_c*|c|^2 term, packed beside cslice
        cnt_pb = np.zeros((CPB, NB), dtype=np.float32)
        cnt_pb[cls_pos, cls_bank] = counts
        cs2 = np.zeros((CS, 520), dtype=np.float32)
        cs2[:, :D] = centers[i * CS : (i + 1) * CS]
        cs2[:, D : D + NB] = cnt_pb

        in_maps.append(
            {
                "feat": featp,
                "cs2": cs2,
                "oh": oh,
                "identity": ident,
                "ctab": ctab,
            }
        )
    return in_maps


def combine(outs):
    """outs: list of 8 [1,32] f32 arrays -> scalar loss (np.float32)."""
    cen = 0.0
    ang = 0.0
    for o in outs:
        o = np.asarray(o, dtype=np.float64)
        cen += o[0, 0:18].sum()  # sum f^2 + counts.|c|^2 - 2*cross
        ang += o[0, 18:20].sum()
    ang -= C * (1.0 - CT) ** 2  # remove the diagonal (sim_ii == 1) terms
    loss = 0.5 * cen / B + ang / (0.5 * C * (C - 1))
    return np.float32(loss)


def kernel(y, feat, centers):
    nc = build()
    in_maps = make_in_maps(y, feat, centers)
    res = run_bass_kernel_spmd(nc, in_maps, core_ids=list(range(N_CORES)))
    return combine([res.results[i]["out"] for i in range(N_CORES)])


# revision 7
# speedup vs baseline: 1.6977x; 1.1557x over previous
"""Trainium2 Bass kernel for AngelLoss (center loss + angular loss).

loss = 0.5*sum((feat - centers[y])^2)/B
     + sum_offdiag((c_i.c_j/(|c_i||c_j|) - ct)^2) / (0.5*C*(C-1))

Sharding (8 NeuronCores, data-parallel over batch):
  - center term, gather-free:  sum||f||^2 - 2*sum_c c_c.S_c + sum_c n_c||c_c||^2
    where S_c = sum of feat rows with label c.  Host buckets each core's
    8192 rows into 8 class-banks (125 classes each, greedy-balanced), pads
    each bank's rows to 9x128, and ships per-row relative class ids.
    gpsimd expands them to onehots (iota compare); S for one bank
    accumulates in one PSUM tile via 9 onehot^T @ feat matmuls and drains
    with a DVE multiply + free-axis reduce against the resident centers.
    sum||f||^2 splits across ScalarE (7/9 slots, square+accumulate) and
    DVE (2/9 slots, multiply + reduce).
  - angular term: center rows sharded (125 rows/core); each core computes
    its 125x1000 slice of the normalized Gram matrix on TensorE.
  - per-core [1,32] partial sums are combined on the host.
"""

from contextlib import ExitStack

import ml_dtypes
import numpy as np

import concourse.bass as bass
import concourse.tile as tile
from concourse import bacc, mybir
from concourse.bass import ds, ts
from concourse.bass_utils import run_bass_kernel_spmd

N_CORES = 8
B, C, D = 65536, 1000, 512
BS = B // N_CORES  # 8192 rows per core
NB = 8  # class banks
CPB = C // NB  # 125 classes per bank
SUB = 9  # 128-row subtiles per bank group (1152 slots >= ~1024+slack rows)
GROUP = 128 * SUB  # 1152
PR = NB * GROUP  # 9216 padded rows per core
CS = C // N_CORES  # 125 gram rows per core
NSC = 8  # feat slots squared on ScalarE; the rest (SUB-NSC) go to DVE

# ct = 2*radius(C-1)^2 - 1 from the reference, evaluated in f64, cast f32.
CT = float(np.float32(-0.0010010010010047532))

_F32 = mybir.dt.float32
_BF16 = mybir.dt.bfloat16

_NC_CACHE = {}


def _angular_tensor(tc, ctx, ptp, pgram, cn_all, cns, cnT, cnTs, ident):
    nc = tc.nc
    for s in range(NB):
        for ki in range(4):
            pt = ptp.tile([128, 128], _BF16, tag="tp")
            nc.tensor.transpose(pt[:, :CPB], cn_all[:, s, ts(ki, 128)], ident[:CPB, :CPB])
            nc.vector.tensor_copy(cnT[:, ki, :, s : s + 1], pt[:, :CPB])
    for ki in range(4):
        pt = ptp.tile([128, 128], _BF16, tag="tp")
        nc.tensor.transpose(pt[:, :CS], cns[:CS, ts(ki, 128)], ident[:CS, :CS])
        nc.vector.tensor_copy(cnTs[:, ki, :], pt[:, :CS])
    HALVES = [(0, 63), (63, 62)]
    grams = []
    for q0, qn in HALVES:
        pg = pgram.tile([CS, 63 * NB], _F32, tag="gram")
        for ki in range(4):
            nc.tensor.matmul(
                pg[:, : qn * NB],
                cnTs[:, ki, :],
                cnT[:, ki, ds(q0, qn), :],
                start=(ki == 0),
                stop=(ki == 3),
            )
        grams.append((pg, qn))
    return grams


def _build_body(ctx, tc, feat, cs2, rel, iotab, identity, cbf, out):
    nc = tc.nc
    AF = mybir.ActivationFunctionType

    const = ctx.enter_context(tc.tile_pool(name="const", bufs=1))
    pnrm = ctx.enter_context(tc.tile_pool(name="nrm", bufs=3))
    pfeat = ctx.enter_context(tc.tile_pool(name="feat", bufs=3))
    poht = ctx.enter_context(tc.tile_pool(name="oht", bufs=3))
    psq = ctx.enter_context(tc.tile_pool(name="sq", bufs=2))
    pdscr = ctx.enter_context(tc.tile_pool(name="dscr", bufs=2))
    ptp = ctx.enter_context(tc.tile_pool(name="tp", bufs=2, space="PSUM"))
    pgram = ctx.enter_context(tc.tile_pool(name="gram", bufs=2, space="PSUM"))
    pS = ctx.enter_context(tc.tile_pool(name="S", bufs=2, space="PSUM"))
    pfin = ctx.enter_context(tc.tile_pool(name="fin", bufs=1, space="PSUM"))

    ident = const.tile([128, 128], _BF16)
    nc.sync.dma_start(ident[:], identity[:, :])
    relt = const.tile([128, NB * SUB, 1], _F32)
    nc.sync.dma_start(relt[:], rel[:, :, :])
    iot = const.tile([128, 1, 128], _F32)
    nc.sync.dma_start(iot[:], iotab[:, :, :])
    ones = const.tile([128, 1], _F32)
    nc.vector.memset(ones[:], 1.0)
    # staging cols: 0-7 scalar sum(f^2); 8 counts.|c|^2; 10-17 vector
    # sum(f^2); 18-25 +cross per bank (host applies -2); 26-27 angular.
    staging = const.tile([128, 32], _F32)
    nc.vector.memset(staging[:], 0.0)
    ctbias = const.tile([128, 1], _F32)
    nc.vector.memset(ctbias[:], -CT)

    # centers table, bank-major: (p, j) holds class bankclasses[j][p]
    ct_all = const.tile([CPB, NB, D], _BF16)
    nc.scalar.dma_start(ct_all[:], cbf.rearrange("(p s) d -> p s d", p=CPB))
    # gram row slice (f32) + per-class counts packed in cols 512:520
    tslice = const.tile([128, 520], _F32)
    nc.sync.dma_start(tslice[:CS, :], cs2[:, :])

    # --- angular norms (ScalarE square+sqrt, DVE reciprocal+normalize) ---
    nsq = const.tile([CPB, NB], _F32)
    for s in range(NB):
        ttr = pnrm.tile([CPB, D], _F32, tag="ttr")
        nc.scalar.activation(
            ttr[:], ct_all[:, s, :], AF.Square, accum_out=nsq[:, s : s + 1]
        )
    sq2 = pnrm.tile([128, D], _F32, tag="sq2")
    nsq2 = pnrm.tile([128, 1], _F32, tag="nsq2")
    nc.scalar.activation(
        sq2[:CS, :], tslice[:CS, :D], AF.Square, accum_out=nsq2[:CS, :]
    )
    sd = pnrm.tile([CPB, NB], _F32, tag="sd")
    nc.scalar.activation(sd[:], nsq[:], AF.Sqrt)
    sd2 = pnrm.tile([128, 1], _F32, tag="sd2")
    nc.scalar.activation(sd2[:CS, :], nsq2[:CS, :], AF.Sqrt)
    inv = pnrm.tile([CPB, NB], _F32, tag="inv")
    nc.vector.reciprocal(inv[:], sd[:])
    inv2 = pnrm.tile([128, 1], _F32, tag="inv2")
    nc.vector.reciprocal(inv2[:CS, :], sd2[:CS, :])
    cn_all = const.tile([CPB, NB, D], _BF16)
    nc.vector.tensor_tensor(
        out=cn_all[:],
        in0=ct_all[:],
        in1=inv[:, :].to_broadcast([CPB, NB, D]),
        op=mybir.AluOpType.mult,
    )
    cns = const.tile([128, D], _BF16)
    nc.vector.tensor_tensor(
        out=cns[:CS, :],
        in0=tslice[:CS, :D],
        in1=inv2[:CS, :].to_broadcast([CS, D]),
        op=mybir.AluOpType.mult,
    )
    cnT = const.tile([128, 4, CPB, NB], _BF16)
    cnTs = const.tile([128, 4, CS], _BF16)

    # counts . |c|^2 term (DVE)
    cscr = pnrm.tile([CPB, NB], _F32, tag="cscr")
    nc.vector.tensor_tensor(
        out=cscr[:],
        in0=nsq[:],
        in1=tslice[:CS, 512:520],
        op=mybir.AluOpType.mult,
    )
    nc.vector.tensor_reduce(
        out=staging[:CS, 8:9],
        in_=cscr[:],
        axis=mybir.AxisListType.X,
        op=mybir.AluOpType.add,
    )

    # --- center-loss main loop: one PSUM scatter bank per group ---
    grams = None
    for g in range(NB):
        ft = pfeat.tile([128, SUB, D], _BF16, tag="ft")
        nc.sync.dma_start(
            ft[:], feat[ds(g * GROUP, GROUP), :].rearrange("(p s) d -> p s d", p=128)
        )
        ohg = poht.tile([128, SUB, 128], _BF16, tag="ohg")
        nc.vector.tensor_tensor(
            out=ohg[:],
            in0=relt[:, ds(g * SUB, SUB), :].to_broadcast([128, SUB, 128]),
            in1=iot[:, :, :].to_broadcast([128, SUB, 128]),
            op=mybir.AluOpType.is_equal,
        )
        sqs = psq.tile([128, NSC, D], _BF16, tag="sqs")
        nc.scalar.activation(
            sqs[:], ft[:, :NSC, :], AF.Square, accum_out=staging[:, g : g + 1]
        )
        sqv = psq.tile([128, SUB - NSC, D], _BF16, tag="sqv")
        nc.vector.tensor_tensor(
            out=sqv[:],
            in0=ft[:, NSC:, :],
            in1=ft[:, NSC:, :],
            op=mybir.AluOpType.mult,
        )
        nc.vector.tensor_reduce(
            out=staging[:, 10 + g : 11 + g],
            in_=sqv[:],
            axis=mybir.AxisListType.XY,
            op=mybir.AluOpType.add,
        )
        st = pS.tile([128, D], _F32, tag="S")
        for s in range(SUB):
            nc.tensor.matmul(
                st[:],
                ohg[:, s, :],
                ft[:, s, :],
                start=(s == 0),
                stop=(s == SUB - 1),
            )
        dscr = pdscr.tile([CPB, D], _F32, tag="dscr")
        nc.vector.tensor_tensor(
            out=dscr[:],
            in0=st[:CPB, :],
            in1=ct_all[:, g, :],
            op=mybir.AluOpType.mult,
        )
        nc.vector.tensor_reduce(
            out=staging[:CPB, 18 + g : 19 + g],
            in_=dscr[:],
            axis=mybir.AxisListType.X,
            op=mybir.AluOpType.add,
        )
        if g == 1:
            # angular transposes + gram slot into the Tensor stream here:
            # cn_all is ready by now and the scatter stream has slack
            grams = _angular_tensor(tc, ctx, ptp, pgram, cn_all, cns, cnT, cnTs, ident)

    # --- angular squares + final partition collapse ---
    for h, (pg, qn) in enumerate(grams):
        gs = pnrm.tile([CS, 63 * NB], _F32, tag="gscr")
        nc.scalar.activation(
            gs[:, : qn * NB],
            pg[:, : qn * NB],
            AF.Square,
            bias=ctbias[:CS, :],
            accum_out=staging[:CS, 26 + h : 27 + h],
        )
    pf = pfin.tile([1, 32], _F32, tag="fin")
    nc.tensor.matmul(pf[:], ones[:], staging[:], start=True, stop=True)
    osb = const.tile([1, 32], _F32)
    nc.vector.tensor_copy(osb[:], pf[:])
    nc.sync.dma_start(out[:, :], osb[:, :])


def build():
    if "nc" in _NC_CACHE:
        return _NC_CACHE["nc"]
    nc = bacc.Bacc(
        "TRN2",
        target_bir_lowering=False,
        debug=False,
        enable_asserts=False,
        num_devices=N_CORES,
    )
    feat = nc.dram_tensor("feat", [PR, D], _BF16, kind="ExternalInput").ap()
    cs2 = nc.dram_tensor("cs2", [CS, 520], _F32, kind="ExternalInput").ap()
    rel = nc.dram_tensor("rel", [128, NB * SUB, 1], _F32, kind="ExternalInput").ap()
    iotab = nc.dram_tensor("iotab", [128, 1, 128], _F32, kind="ExternalInput").ap()
    identity = nc.dram_tensor("identity", [128, 128], _BF16, kind="ExternalInput").ap()
    cbf = nc.dram_tensor("ctab", [C, D], _BF16, kind="ExternalInput").ap()
    out = nc.dram_tensor("out", [1, 32], _F32, kind="ExternalOutput").ap()
    with tile.TileContext(nc) as tc, ExitStack() as ctx:
        _build_body(ctx, tc, feat, cs2, rel, iotab, identity, cbf, out)
    nc.compile()
    _NC_CACHE["nc"] = nc
    return nc


def _bank_assignment(y):
    """Greedy-balanced partition of the C classes into NB banks of CPB each.

    Returns (bankclasses [NB, CPB] int array, class->(bank, pos) maps).
    """
    counts = np.bincount(y, minlength=C)
    order = np.argsort(-counts, kind="stable")
    bank_tot = np.zeros(NB, dtype=np.int64)
    bank_n = np.zeros(NB, dtype=np.int64)
    bankclasses = np.zeros((NB, CPB), dtype=np.int64)
    cls_bank = np.zeros(C, dtype=np.int64)
    cls_pos = np.zeros(C, dtype=np.int64)
    for c in order:
        open_banks = np.flatnonzero(bank_n < CPB)
        j = open_banks[np.argmin(bank_tot[open_banks])]
        bankclasses[j, bank_n[j]] = c
        cls_bank[c] = j
        cls_pos[c] = bank_n[j]
        bank_n[j] += 1
        bank_tot[j] += counts[c]
    assert bank_tot.max() <= GROUP, f"bank overflow: {bank_tot.max()} > {GROUP}"
    return bankclasses, cls_bank, cls_pos, counts


def make_in_maps(y, feat, centers):
    feat = np.ascontiguousarray(feat, dtype=np.float32)
    centers = np.ascontiguousarray(centers, dtype=np.float32)
    y = np.asarray(y).astype(np.int64)
    ident = np.eye(128, dtype=ml_dtypes.bfloat16)
    iotab = np.broadcast_to(
        np.arange(128, dtype=np.float32)[None, None, :], (128, 1, 128)
    ).copy()
    in_maps = []
    for i in range(N_CORES):
        ys = y[i * BS : (i + 1) * BS]
        fs = feat[i * BS : (i + 1) * BS]
        bankclasses, cls_bank, cls_pos, counts = _bank_assignment(ys)

        # bank-major centers table: dram row r = class bankclasses[r % NB][r // NB]
        perm_tab = bankclasses[np.arange(C) % NB, np.arange(C) // NB]
        ctab = centers[perm_tab].astype(ml_dtypes.bfloat16)

        # bucket rows by bank; group g rows sit at slots [g*GROUP, g*GROUP+n_g)
        row_bank = cls_bank[ys]
        grp_order = np.argsort(row_bank, kind="stable")  # rows sorted by bank
        n_per = np.bincount(row_bank, minlength=NB)
        starts = np.zeros(NB + 1, dtype=np.int64)
        starts[1:] = np.cumsum(n_per)
        slot = np.full(PR, -1, dtype=np.int64)  # slot -> source row
        for g in range(NB):
            rows = grp_order[starts[g] : starts[g + 1]]
            slot[g * GROUP : g * GROUP + len(rows)] = rows

        featp = np.zeros((PR, D), dtype=ml_dtypes.bfloat16)
        valid = slot >= 0
        featp[valid] = fs[slot[valid]].astype(ml_dtypes.bfloat16)

        # rel[p, 9g+s] = onehot column of the row at (group g, part p, slot s)
        # i.e. padded row index g*GROUP + 9p + s; -1 (matches nothing) for pads
        rel = np.full((128, NB * SUB, 1), -1.0, dtype=np.float32)
        k = np.flatnonzero(valid)
        g_k = k // GROUP
        r_k = k % GROUP
        p_k = r_k // SUB
        s_k = r_k % SUB
        rel[p_k, g_k * SUB + s_k, 0] = cls_pos[ys[slot[k]]]

        # per-(pos, bank) counts for the n_c*|c|^2 term, packed beside cslice
        cnt_pb = np.zeros((CPB, NB), dtype=np.float32)
        cnt_pb[cls_pos, cls_bank] = counts
        cs2 = np.zeros((CS, 520), dtype=np.float32)
        cs2[:, :D] = centers[i * CS : (i + 1) * CS]
        cs2[:, D : D + NB] = cnt_pb

        in_maps.append(
            {
                "feat": featp,
                "cs2": cs2,
                "rel": rel,
                "iotab": iotab,
                "identity": ident,
                "ctab": ctab,
            }
        )
    return in_maps


def combine(outs):
    """outs: list of 8 [1,32] f32 arrays -> scalar loss (np.float32)."""
    cen = 0.0
    ang = 0.0
    for o in outs:
        o = np.asarray(o, dtype=np.float64)
        cen += o[0, 0:18].sum() - 2.0 * o[0, 18:26].sum()
        ang += o[0, 26:28].sum()
    ang -= C * (1.0 - CT) ** 2  # remove the diagonal (sim_ii == 1) terms
    loss = 0.5 * cen / B + ang / (0.5 * C * (C - 1))
    return np.float32(loss)


def kernel(y, feat, centers):
    nc = build()
    in_maps = make_in_maps(y, feat, centers)
    res = run_bass_kernel_spmd(nc, in_maps, core_ids=list(range(N_CORES)))
    return combine([res.results[i]["out"] for i in range(N_CORES)])


# revision 9
# speedup vs baseline: 1.7706x; 1.0429x over previous
"""Trainium2 Bass kernel for AngelLoss (center loss + angular loss).

loss = 0.5*sum((feat - centers[y])^2)/B
     + sum_offdiag((c_i.c_j/(|c_i||c_j|) - ct)^2) / (0.5*C*(C-1))

Sharding (8 NeuronCores, data-parallel over batch):
  - center term, gather-free:  sum||f||^2 - 2*sum_c c_c.S_c + sum_c n_c||c_c||^2
    where S_c = sum of feat rows with label c.  Host buckets each core's
    8192 rows into 8 class-banks (125 classes each, greedy-balanced), pads
    each bank's rows to 9x128, and ships per-row relative class ids that
    DVE expands to onehots (iota compare).  S for one bank accumulates in
    one PSUM tile via 9 onehot^T @ feat matmuls and drains with a DVE
    multiply + free-axis reduce against the resident centers table.
    sum||f||^2 splits across ScalarE (7/9 slots) and DVE (2/9 slots).
  - angular term via the Frobenius identity (N = row-normalized centers):
      sum_ij (sim-ct)^2 = ||N^T N||_F^2 - 2ct ||sum_i N_i||^2 + C^2 ct^2
    computed redundantly on every core from the resident normalized
    table with 32 accumulating matmuls into a [512,512] PSUM Gram.
  - per-core [1,32] partial sums are combined on the host.
"""

from contextlib import ExitStack

import ml_dtypes
import numpy as np

import concourse.bass as bass
import concourse.tile as tile
from concourse import bacc, mybir
from concourse.bass import ds, ts
from concourse.bass_utils import run_bass_kernel_spmd

N_CORES = 8
B, C, D = 65536, 1000, 512
BS = B // N_CORES  # 8192 rows per core
NB = 8  # class banks
CPB = C // NB  # 125 classes per bank
SUB = 9  # 128-row subtiles per bank group (1152 slots >= ~1024+slack rows)
GROUP = 128 * SUB  # 1152
PR = NB * GROUP  # 9216 padded rows per core
NSC = 7  # feat slots squared on ScalarE; the rest (SUB-NSC) go to DVE

# ct = 2*radius(C-1)^2 - 1 from the reference, evaluated in f64, cast f32.
CT = float(np.float32(-0.0010010010010047532))

_F32 = mybir.dt.float32
_BF16 = mybir.dt.bfloat16

_NC_CACHE = {}


def _build_body(ctx, tc, feat, cnt, rel, iotab, cbf, nbf, out):
    nc = tc.nc
    AF = mybir.ActivationFunctionType

    const = ctx.enter_context(tc.tile_pool(name="const", bufs=1))
    pnrm = ctx.enter_context(tc.tile_pool(name="nrm", bufs=3))
    pfeat = ctx.enter_context(tc.tile_pool(name="feat", bufs=3))
    psq = ctx.enter_context(tc.tile_pool(name="sq", bufs=2))
    pdscr = ctx.enter_context(tc.tile_pool(name="dscr", bufs=2))
    pG = ctx.enter_context(tc.tile_pool(name="G", bufs=1, space="PSUM"))
    pS = ctx.enter_context(tc.tile_pool(name="S", bufs=2, space="PSUM"))
    pfin = ctx.enter_context(tc.tile_pool(name="fin", bufs=1, space="PSUM"))

    # scalar-queue loads: small index tensors first, then the two tables
    relt = const.tile([128, NB * SUB, 1], _F32)
    nc.scalar.dma_start(relt[:], rel[:, :, :])
    iot = const.tile([128, 1, 128], _F32)
    nc.scalar.dma_start(iot[:], iotab[:, :, :])
    ct_all = const.tile([CPB, NB, D], _BF16)
    nc.scalar.dma_start(ct_all[:], cbf.rearrange("(p s) d -> p s d", p=CPB))
    nt_all = const.tile([CPB, NB, D], _BF16)
    nc.scalar.dma_start(nt_all[:], nbf.rearrange("(p s) d -> p s d", p=CPB))
    cntt = const.tile([CPB, NB], _F32)
    nc.scalar.dma_start(cntt[:], cnt[:, :])

    ones = const.tile([128, 1], _F32)
    nc.vector.memset(ones[:], 1.0)
    onesb = const.tile([128, 1], _BF16)
    nc.vector.memset(onesb[:], 1.0)
    # staging cols: 0-7 scalar sum(f^2); 8 counts.|c|^2; 9 ||G||^2;
    # 10-17 vector sum(f^2); 18-25 +cross per bank (host applies -2);
    # 26 ||colsum||^2.
    staging = const.tile([128, 32], _F32)
    nc.vector.memset(staging[:], 0.0)

    # one-shot onehot expansion for all 72 subtiles (DVE iota compare)
    oht = const.tile([128, NB * SUB, 128], _BF16)
    nc.vector.tensor_tensor(
        out=oht[:],
        in0=relt[:, :, :].to_broadcast([128, NB * SUB, 128]),
        in1=iot[:, :, :].to_broadcast([128, NB * SUB, 128]),
        op=mybir.AluOpType.is_equal,
    )

    # --- center-loss main loop: one PSUM scatter bank per group ---
    for g in range(NB):
        ft = pfeat.tile([128, SUB, D], _BF16, tag="ft")
        nc.sync.dma_start(
            ft[:], feat[ds(g * GROUP, GROUP), :].rearrange("(p s) d -> p s d", p=128)
        )
        sqs = psq.tile([128, NSC, D], _BF16, tag="sqs")
        nc.scalar.activation(
            sqs[:], ft[:, :NSC, :], AF.Square, accum_out=staging[:, g : g + 1]
        )
        sqv = psq.tile([128, SUB - NSC, D], _BF16, tag="sqv")
        nc.vector.tensor_tensor(
            out=sqv[:],
            in0=ft[:, NSC:, :],
            in1=ft[:, NSC:, :],
            op=mybir.AluOpType.mult,
        )
        nc.vector.tensor_reduce(
            out=staging[:, 10 + g : 11 + g],
            in_=sqv[:],
            axis=mybir.AxisListType.XY,
            op=mybir.AluOpType.add,
        )
        st = pS.tile([128, D], _F32, tag="S")
        for s in range(SUB):
            nc.tensor.matmul(
                st[:],
                oht[:, g * SUB + s, :],
                ft[:, s, :],
                start=(s == 0),
                stop=(s == SUB - 1),
            )
        dscr = pdscr.tile([CPB, D], _F32, tag="dscr")
        nc.vector.tensor_tensor(
            out=dscr[:],
            in0=st[:CPB, :],
            in1=ct_all[:, g, :],
            op=mybir.AluOpType.mult,
        )
        nc.vector.tensor_reduce(
            out=staging[:CPB, 18 + g : 19 + g],
            in_=dscr[:],
            axis=mybir.AxisListType.X,
            op=mybir.AluOpType.add,
        )
        if g == 2:
            # angular Gram + column-sum slot into the Tensor stream here:
            # the normalized table is resident by now and the scatter
            # stream has slack
            Gt = pG.tile([128, 4, D], _F32, tag="G")
            for j in range(NB):
                for ki in range(4):
                    nc.tensor.matmul(
                        Gt[:, ki, :],
                        nt_all[:, j, ts(ki, 128)],
                        nt_all[:, j, :],
                        start=(j == 0),
                        stop=(j == NB - 1),
                    )
            csf = pG.tile([1, D], _F32, tag="cs")
            for j in range(NB):
                nc.tensor.matmul(
                    csf[0:1, :],
                    onesb[:CPB, :],
                    nt_all[:, j, :],
                    start=(j == 0),
                    stop=(j == NB - 1),
                )

    # --- angular squares + counts + final partition collapse ---
    gsq = pnrm.tile([128, 4, D], _F32, tag="gsq")
    nc.scalar.activation(gsq[:], Gt[:], AF.Square, accum_out=staging[:, 9:10])
    css = pnrm.tile([1, D], _F32, tag="css")
    nc.scalar.activation(css[0:1, :], csf[0:1, :], AF.Square, accum_out=staging[0:1, 26:27])
    # counts . |c|^2: per-class squared norms via ScalarE then DVE dot
    nsq = const.tile([CPB, NB], _F32)
    for s in range(NB):
        ttr = pnrm.tile([CPB, D], _F32, tag="ttr")
        nc.scalar.activation(
            ttr[:], ct_all[:, s, :], AF.Square, accum_out=nsq[:, s : s + 1]
        )
    cscr = pnrm.tile([CPB, NB], _F32, tag="cscr")
    nc.vector.tensor_tensor(
        out=cscr[:], in0=nsq[:], in1=cntt[:, :], op=mybir.AluOpType.mult
    )
    nc.vector.tensor_reduce(
        out=staging[:CPB, 8:9],
        in_=cscr[:],
        axis=mybir.AxisListType.X,
        op=mybir.AluOpType.add,
    )

    pf = pfin.tile([1, 32], _F32, tag="fin")
    nc.tensor.matmul(pf[:], ones[:], staging[:], start=True, stop=True)
    osb = const.tile([1, 32], _F32)
    nc.vector.tensor_copy(osb[:], pf[:])
    nc.sync.dma_start(out[:, :], osb[:, :])


def build():
    if "nc" in _NC_CACHE:
        return _NC_CACHE["nc"]
    nc = bacc.Bacc(
        "TRN2",
        target_bir_lowering=False,
        debug=False,
        enable_asserts=False,
        num_devices=N_CORES,
    )
    feat = nc.dram_tensor("feat", [PR, D], _BF16, kind="ExternalInput").ap()
    cnt = nc.dram_tensor("cnt", [CPB, NB], _F32, kind="ExternalInput").ap()
    rel = nc.dram_tensor("rel", [128, NB * SUB, 1], _F32, kind="ExternalInput").ap()
    iotab = nc.dram_tensor("iotab", [128, 1, 128], _F32, kind="ExternalInput").ap()
    cbf = nc.dram_tensor("ctab", [C, D], _BF16, kind="ExternalInput").ap()
    nbf = nc.dram_tensor("ntab", [C, D], _BF16, kind="ExternalInput").ap()
    out = nc.dram_tensor("out", [1, 32], _F32, kind="ExternalOutput").ap()
    with tile.TileContext(nc) as tc, ExitStack() as ctx:
        _build_body(ctx, tc, feat, cnt, rel, iotab, cbf, nbf, out)
    nc.compile()
    _NC_CACHE["nc"] = nc
    return nc


def _bank_assignment(y):
    """Greedy-balanced partition of the C classes into NB banks of CPB each."""
    counts = np.bincount(y, minlength=C)
    order = np.argsort(-counts, kind="stable")
    bank_tot = np.zeros(NB, dtype=np.int64)
    bank_n = np.zeros(NB, dtype=np.int64)
    bankclasses = np.zeros((NB, CPB), dtype=np.int64)
    cls_bank = np.zeros(C, dtype=np.int64)
    cls_pos = np.zeros(C, dtype=np.int64)
    for c in order:
        open_banks = np.flatnonzero(bank_n < CPB)
        j = open_banks[np.argmin(bank_tot[open_banks])]
        bankclasses[j, bank_n[j]] = c
        cls_bank[c] = j
        cls_pos[c] = bank_n[j]
        bank_n[j] += 1
        bank_tot[j] += counts[c]
    assert bank_tot.max() <= GROUP, f"bank overflow: {bank_tot.max()} > {GROUP}"
    return bankclasses, cls_bank, cls_pos, counts


def make_in_maps(y, feat, centers):
    feat = np.ascontiguousarray(feat, dtype=np.float32)
    centers = np.ascontiguousarray(centers, dtype=np.float32)
    y = np.asarray(y).astype(np.int64)
    iotab = np.broadcast_to(
        np.arange(128, dtype=np.float32)[None, None, :], (128, 1, 128)
    ).copy()
    norm = np.linalg.norm(centers.astype(np.float64), axis=1, keepdims=True)
    ncenters = (centers / norm).astype(ml_dtypes.bfloat16)
    in_maps = []
    for i in range(N_CORES):
        ys = y[i * BS : (i + 1) * BS]
        fs = feat[i * BS : (i + 1) * BS]
        bankclasses, cls_bank, cls_pos, counts = _bank_assignment(ys)

        # bank-major tables: dram row r = class bankclasses[r % NB][r // NB]
        perm_tab = bankclasses[np.arange(C) % NB, np.arange(C) // NB]
        ctab = centers[perm_tab].astype(ml_dtypes.bfloat16)
        ntab = ncenters[perm_tab]

        # bucket rows by bank; group g rows sit at slots [g*GROUP, g*GROUP+n_g)
        row_bank = cls_bank[ys]
        grp_order = np.argsort(row_bank, kind="stable")
        n_per = np.bincount(row_bank, minlength=NB)
        starts = np.zeros(NB + 1, dtype=np.int64)
        starts[1:] = np.cumsum(n_per)
        slot = np.full(PR, -1, dtype=np.int64)  # slot -> source row
        for g in range(NB):
            rows = grp_order[starts[g] : starts[g + 1]]
            slot[g * GROUP : g * GROUP + len(rows)] = rows

        featp = np.zeros((PR, D), dtype=ml_dtypes.bfloat16)
        valid = slot >= 0
        featp[valid] = fs[slot[valid]].astype(ml_dtypes.bfloat16)

        # rel[p, 9g+s] = onehot column of the row at (group g, part p, slot s)
        # i.e. padded row index g*GROUP + 9p + s; -1 (matches nothing) for pads
        rel = np.full((128, NB * SUB, 1), -1.0, dtype=np.float32)
        k = np.flatnonzero(valid)
        g_k = k // GROUP
        r_k = k % GROUP
        p_k = r_k // SUB
        s_k = r_k % SUB
        rel[p_k, g_k * SUB + s_k, 0] = cls_pos[ys[slot[k]]]

        cnt_pb = np.zeros((CPB, NB), dtype=np.float32)
        cnt_pb[cls_pos, cls_bank] = counts

        in_maps.append(
            {
                "feat": featp,
                "cnt": cnt_pb,
                "rel": rel,
                "iotab": iotab,
                "ctab": ctab,
                "ntab": ntab,
            }
        )
    return in_maps


def combine(outs):
    """outs: list of 8 [1,32] f32 arrays -> scalar loss (np.float32)."""
    cen = 0.0
    for o in outs:
        o = np.asarray(o, dtype=np.float64)
        cen += o[0, 0:9].sum() + o[0, 10:18].sum() - 2.0 * o[0, 18:26].sum()
    o0 = np.asarray(outs[0], dtype=np.float64)
    gsq, ssq = o0[0, 9], o0[0, 26]
    ang = gsq - 2.0 * CT * ssq + C * C * CT * CT - C * (1.0 - CT) ** 2
    loss = 0.5 * cen / B + ang / (0.5 * C * (C - 1))
    return np.float32(loss)


def kernel(y, feat, centers):
    nc = build()
    in_maps = make_in_maps(y, feat, centers)
    res = run_bass_kernel_spmd(nc, in_maps, core_ids=list(range(N_CORES)))
    return combine([res.results[i]["out"] for i in range(N_CORES)])


# revision 23
# speedup vs baseline: 1.8507x; 1.0453x over previous
"""Trainium2 Bass kernel for AngelLoss (center loss + angular loss).

loss = 0.5*sum((feat - centers[y])^2)/B
     + sum_offdiag((c_i.c_j/(|c_i||c_j|) - ct)^2) / (0.5*C*(C-1))

Sharding (8 NeuronCores, data-parallel over batch):
  - center term, gather-free:  sum||f||^2 - 2*sum_c c_c.S_c + sum_c n_c||c_c||^2
    where S_c = sum of feat rows with label c.  Host buckets each core's
    8192 rows into 8 class-banks (125 classes each, greedy-balanced), pads
    each bank's rows to 9x128, and ships per-row relative class ids that
    DVE expands to onehots (iota compare).  S for two banks accumulates in
    one 2-bank PSUM tile via 2x9 onehot^T @ feat matmuls and drains with
    one DVE multiply + free-axis reduce against the resident centers.
    sum||f||^2 runs on ScalarE (square + free-axis accumulate).
  - angular term via the Frobenius identity (N = row-normalized centers):
      sum_ij (sim-ct)^2 = ||N^T N||_F^2 - 2ct ||sum_i N_i||^2 + C^2 ct^2
    computed redundantly on every core from the resident normalized
    table with 32 accumulating matmuls into a [512,512] PSUM Gram.
  - per-core [1,32] partial sums are combined on the host.
"""

from contextlib import ExitStack

import ml_dtypes
import numpy as np

import concourse.bass as bass
import concourse.tile as tile
from concourse import bacc, mybir
from concourse.bass import ds, ts
from concourse.bass_utils import run_bass_kernel_spmd

N_CORES = 8
B, C, D = 65536, 1000, 512
BS = B // N_CORES  # 8192 rows per core
NB = 8  # class banks
CPB = C // NB  # 125 classes per bank
SUB = 9  # 128-row subtiles per bank group (1152 slots >= ~1024+slack rows)
GROUP = 128 * SUB  # 1152
PR = NB * GROUP  # 9216 padded rows per core

# ct = 2*radius(C-1)^2 - 1 from the reference, evaluated in f64, cast f32.
CT = float(np.float32(-0.0010010010010047532))

_F32 = mybir.dt.float32
_BF16 = mybir.dt.bfloat16

_NC_CACHE = {}


def _build_body(ctx, tc, feat, cnt, rel, iotab, cbf, nbf, out):
    nc = tc.nc
    AF = mybir.ActivationFunctionType

    const = ctx.enter_context(tc.tile_pool(name="const", bufs=1))
    pnrm = ctx.enter_context(tc.tile_pool(name="nrm", bufs=2))
    pfeat = ctx.enter_context(tc.tile_pool(name="feat", bufs=3))
    psq = ctx.enter_context(tc.tile_pool(name="sq", bufs=2))
    pdscr = ctx.enter_context(tc.tile_pool(name="dscr", bufs=2))
    poht = ctx.enter_context(tc.tile_pool(name="oht", bufs=2))
    pG = ctx.enter_context(tc.tile_pool(name="G", bufs=1, space="PSUM"))
    pS = ctx.enter_context(tc.tile_pool(name="S", bufs=2, space="PSUM"))

    # scalar-queue loads: small index tensor first, then the two tables
    relt = const.tile([128, NB * SUB, 1], _BF16)
    nc.scalar.dma_start(relt[:], rel[:, :, :])
    ct_all = const.tile([CPB, NB, D], _BF16)
    nc.scalar.dma_start(ct_all[:], cbf.rearrange("(p s) d -> p s d", p=CPB))
    nt_all = const.tile([CPB, NB, D], _BF16)
    nc.scalar.dma_start(nt_all[:], nbf.rearrange("(p s) d -> p s d", p=CPB))
    cntt = const.tile([CPB, NB, 2], _F32)
    nc.scalar.dma_start(cntt[:], cnt[:, :, :])

    iot = const.tile([128, 1, 128], _BF16)
    nc.scalar.dma_start(iot[:], iotab[:, :, :])

    ones = const.tile([128, 1], _F32)
    nc.vector.memset(ones[:], 1.0)
    onesb = const.tile([128, 1], _BF16)
    nc.vector.memset(onesb[:], 1.0)
    # staging cols: 0-7 sum(f^2) per group; 8 counts.|c|^2; 9 ||G||^2;
    # 10-13 +cross per bank pair (host applies -2); 14 ||colsum||^2.
    staging = const.tile([128, 16], _F32)
    nc.vector.memset(staging[:], 0.0)

    # counts . |c|^2 (norms^2 host-packed beside the counts)
    cscr = pnrm.tile([CPB, NB], _F32, tag="cscr")
    nc.vector.tensor_tensor(
        out=cscr[:], in0=cntt[:, :, 0], in1=cntt[:, :, 1], op=mybir.AluOpType.mult
    )
    nc.vector.tensor_reduce(
        out=staging[:CPB, 8:9],
        in_=cscr[:],
        axis=mybir.AxisListType.X,
        op=mybir.AluOpType.add,
    )

    # --- center-loss main loop; S PSUM tile covers two banks ---
    Gt = None
    for g in range(NB):
        if g % 2 == 0:
            ohg = poht.tile([128, 2 * SUB, 128], _BF16, tag="ohg")
            nc.vector.tensor_tensor(
                out=ohg[:],
                in0=relt[:, ds(g * SUB, 2 * SUB), :].to_broadcast([128, 2 * SUB, 128]),
                in1=iot[:, :, :].to_broadcast([128, 2 * SUB, 128]),
                op=mybir.AluOpType.is_equal,
            )
            st = pS.tile([128, 2, D], _F32, tag="S")
        ft = pfeat.tile([128, SUB, D], _BF16, tag="ft")
        nc.sync.dma_start(
            ft[:], feat[ds(g * GROUP, GROUP), :].rearrange("(p s) d -> p s d", p=128)
        )
        sqs = psq.tile([128, SUB, D], _BF16, tag="sqs")
        nc.scalar.activation(
            sqs[:], ft[:], AF.Square, accum_out=staging[:, g : g + 1]
        )
        for s in range(SUB):
            nc.tensor.matmul(
                st[:, g % 2, :],
                ohg[:, (g % 2) * SUB + s, :],
                ft[:, s, :],
                start=(s == 0),
                stop=(s == SUB - 1),
            )
        if g % 2 == 1:
            dscr = pdscr.tile([CPB, 2, D], _F32, tag="dscr")
            nc.vector.tensor_tensor(
                out=dscr[:],
                in0=st[:CPB, :, :],
                in1=ct_all[:, ds(g - 1, 2), :],
                op=mybir.AluOpType.mult,
            )
            nc.vector.tensor_reduce(
                out=staging[:CPB, 10 + g // 2 : 11 + g // 2],
                in_=dscr[:],
                axis=mybir.AxisListType.XY,
                op=mybir.AluOpType.add,
            )
        if g in (2, 4):
            # angular Gram (two 2-bank passes) slots into the Tensor
            # stream here: the normalized table is resident and the
            # scatter stream has slack
            kis = (0, 1) if g == 2 else (2, 3)
            Gt = pG.tile([128, 2, D], _F32, tag="G")
            for j in range(NB):
                for kx, ki in enumerate(kis):
                    nc.tensor.matmul(
                        Gt[:, kx, :],
                        nt_all[:, j, ts(ki, 128)],
                        nt_all[:, j, :],
                        start=(j == 0),
                        stop=(j == NB - 1),
                    )
            col = 9 if g == 2 else 15
            gsq = pnrm.tile([128, 2, D], _F32, tag="gsq")
            nc.scalar.activation(
                gsq[:], Gt[:], AF.Square, accum_out=staging[:, col : col + 1]
            )

    # colsum of the normalized table (after ||G||^2 freed the Gram banks)
    csf = pG.tile([1, D], _F32, tag="cs")
    for j in range(NB):
        nc.tensor.matmul(
            csf[0:1, :],
            onesb[:CPB, :],
            nt_all[:, j, :],
            start=(j == 0),
            stop=(j == NB - 1),
        )
    css = pnrm.tile([1, D], _F32, tag="css")
    nc.scalar.activation(
        css[0:1, :], csf[0:1, :], AF.Square, accum_out=staging[0:1, 14:15]
    )

    pf = pG.tile([1, 16], _F32, tag="cs")
    nc.tensor.matmul(pf[:], ones[:], staging[:], start=True, stop=True)
    osb = const.tile([1, 16], _F32)
    nc.vector.tensor_copy(osb[:], pf[:])
    nc.sync.dma_start(out[:, :], osb[:, :])


def build():
    if "nc" in _NC_CACHE:
        return _NC_CACHE["nc"]
    nc = bacc.Bacc(
        "TRN2",
        target_bir_lowering=False,
        debug=False,
        enable_asserts=False,
        num_devices=N_CORES,
    )
    feat = nc.dram_tensor("feat", [PR, D], _BF16, kind="ExternalInput").ap()
    cnt = nc.dram_tensor("cnt", [CPB, NB, 2], _F32, kind="ExternalInput").ap()
    rel = nc.dram_tensor("rel", [128, NB * SUB, 1], _BF16, kind="ExternalInput").ap()
    iotab = nc.dram_tensor("iotab", [128, 1, 128], _BF16, kind="ExternalInput").ap()
    cbf = nc.dram_tensor("ctab", [C, D], _BF16, kind="ExternalInput").ap()
    nbf = nc.dram_tensor("ntab", [C, D], _BF16, kind="ExternalInput").ap()
    out = nc.dram_tensor("out", [1, 16], _F32, kind="ExternalOutput").ap()
    with tile.TileContext(nc) as tc, ExitStack() as ctx:
        _build_body(ctx, tc, feat, cnt, rel, iotab, cbf, nbf, out)
    nc.compile()
    _NC_CACHE["nc"] = nc
    return nc


def _bank_assignment(y):
    """Greedy-balanced partition of the C classes into NB banks of CPB each."""
    counts = np.bincount(y, minlength=C)
    order = np.argsort(-counts, kind="stable")
    bank_tot = np.zeros(NB, dtype=np.int64)
    bank_n = np.zeros(NB, dtype=np.int64)
    bankclasses = np.zeros((NB, CPB), dtype=np.int64)
    cls_bank = np.zeros(C, dtype=np.int64)
    cls_pos = np.zeros(C, dtype=np.int64)
    for c in order:
        open_banks = np.flatnonzero(bank_n < CPB)
        j = open_banks[np.argmin(bank_tot[open_banks])]
        bankclasses[j, bank_n[j]] = c
        cls_bank[c] = j
        cls_pos[c] = bank_n[j]
        bank_n[j] += 1
        bank_tot[j] += counts[c]
    assert bank_tot.max() <= GROUP, f"bank overflow: {bank_tot.max()} > {GROUP}"
    return bankclasses, cls_bank, cls_pos, counts


def make_in_maps(y, feat, centers):
    feat = np.ascontiguousarray(feat, dtype=np.float32)
    centers = np.ascontiguousarray(centers, dtype=np.float32)
    y = np.asarray(y).astype(np.int64)
    norm2 = np.sum(centers.astype(np.float64) ** 2, axis=1, keepdims=True)
    ncenters = (centers / np.sqrt(norm2)).astype(ml_dtypes.bfloat16)
    iotab = np.broadcast_to(
        np.arange(128, dtype=np.float32)[None, None, :], (128, 1, 128)
    ).astype(ml_dtypes.bfloat16)
    in_maps = []
    for i in range(N_CORES):
        ys = y[i * BS : (i + 1) * BS]
        fs = feat[i * BS : (i + 1) * BS]
        bankclasses, cls_bank, cls_pos, counts = _bank_assignment(ys)

        # bank-major tables: dram row r = class bankclasses[r % NB][r // NB]
        perm_tab = bankclasses[np.arange(C) % NB, np.arange(C) // NB]
        ctab = centers[perm_tab].astype(ml_dtypes.bfloat16)
        ntab = ncenters[perm_tab]

        # bucket rows by bank; group g rows sit at slots [g*GROUP, g*GROUP+n_g)
        row_bank = cls_bank[ys]
        grp_order = np.argsort(row_bank, kind="stable")
        n_per = np.bincount(row_bank, minlength=NB)
        starts = np.zeros(NB + 1, dtype=np.int64)
        starts[1:] = np.cumsum(n_per)
        slot = np.full(PR, -1, dtype=np.int64)  # slot -> source row
        for g in range(NB):
            rows = grp_order[starts[g] : starts[g + 1]]
            slot[g * GROUP : g * GROUP + len(rows)] = rows

        featp = np.zeros((PR, D), dtype=ml_dtypes.bfloat16)
        valid = slot >= 0
        featp[valid] = fs[slot[valid]].astype(ml_dtypes.bfloat16)

        # rel[p, 9g+s] = onehot column of the row at (group g, part p, slot s)
        # i.e. padded row index g*GROUP + 9p + s; -1 (matches nothing) for pads
        rel = np.full((128, NB * SUB, 1), -1.0, dtype=ml_dtypes.bfloat16)
        k = np.flatnonzero(valid)
        g_k = k // GROUP
        r_k = k % GROUP
        p_k = r_k // SUB
        s_k = r_k % SUB
        rel[p_k, g_k * SUB + s_k, 0] = cls_pos[ys[slot[k]]]

        cnt_pb = np.zeros((CPB, NB, 2), dtype=np.float32)
        cnt_pb[cls_pos, cls_bank, 0] = counts
        cnt_pb[cls_pos, cls_bank, 1] = norm2[:, 0]

        in_maps.append(
            {
                "feat": featp,
                "cnt": cnt_pb,
                "rel": rel,
                "iotab": iotab,
                "ctab": ctab,
                "ntab": ntab,
            }
        )
    return in_maps


def combine(outs):
    """outs: list of 8 [1,16] f32 arrays -> scalar loss (np.float32)."""
    cen = 0.0
    for o in outs:
        o = np.asarray(o, dtype=np.float64)
        cen += o[0, 0:9].sum() - 2.0 * o[0, 10:14].sum()
    o0 = np.asarray(outs[0], dtype=np.float64)
    gsq, ssq = o0[0, 9] + o0[0, 15], o0[0, 14]
    ang = gsq - 2.0 * CT * ssq + C * C * CT * CT - C * (1.0 - CT) ** 2
    loss = 0.5 * cen / B + ang / (0.5 * C * (C - 1))
    return np.float32(loss)


def kernel(y, feat, centers):
    nc = build()
    in_maps = make_in_maps(y, feat, centers)
    res = run_bass_kernel_spmd(nc, in_maps, core_ids=list(range(N_CORES)))
    return combine([res.results[i]["out"] for i in range(N_CORES)])


# revision 25
# speedup vs baseline: 2.5364x; 1.3705x over previous
"""Trainium2 Bass kernel for AngelLoss (center loss + angular loss).

loss = 0.5*sum((feat - centers[y])^2)/B
     + sum_offdiag((c_i.c_j/(|c_i||c_j|) - ct)^2) / (0.5*C*(C-1))

Sharding (8 NeuronCores, data-parallel over batch):
  - center term, gather-free:  sum||f||^2 - 2*sum_c c_c.S_c + sum_c n_c||c_c||^2
    where S_c = sum of feat rows with label c.  Host buckets each core's
    8192 rows into 8 class-banks (125 classes each, greedy-balanced), pads
    each bank's rows to 9x128 and ships them fp8; per-row relative class
    ids expand to fp8 onehots on DVE (iota compare).  S for two banks
    accumulates in one 2-bank PSUM tile via 2x9 onehot^T @ feat fp8
    matmuls and drains with one DVE multiply + free-axis reduce against
    the resident bf16 centers.  sum||f^2|| runs on ScalarE.
  - angular term via the Frobenius identity (N = row-normalized centers):
      sum_ij (sim-ct)^2 = ||N^T N||_F^2 - 2ct ||sum_i N_i||^2 + C^2 ct^2
    computed redundantly on every core from the resident normalized
    table with 2x16 accumulating matmuls into 2-bank PSUM Gram halves.
  - per-core [1,16] partial sums are combined on the host.
"""

from contextlib import ExitStack

import ml_dtypes
import numpy as np

import concourse.bass as bass
import concourse.tile as tile
from concourse import bacc, mybir
from concourse.bass import ds, ts
from concourse.bass_utils import run_bass_kernel_spmd

N_CORES = 8
B, C, D = 65536, 1000, 512
BS = B // N_CORES  # 8192 rows per core
NB = 8  # class banks
CPB = C // NB  # 125 classes per bank
SUB = 9  # 128-row subtiles per bank group (1152 slots >= ~1024+slack rows)
GROUP = 128 * SUB  # 1152
PR = NB * GROUP  # 9216 padded rows per core

# ct = 2*radius(C-1)^2 - 1 from the reference, evaluated in f64, cast f32.
CT = float(np.float32(-0.0010010010010047532))

_F32 = mybir.dt.float32
_BF16 = mybir.dt.bfloat16
_FP8 = mybir.dt.float8e4

_NC_CACHE = {}


def _build_body(ctx, tc, feat, cnt, sml, cbf, nbf, out):
    nc = tc.nc
    AF = mybir.ActivationFunctionType

    const = ctx.enter_context(tc.tile_pool(name="const", bufs=1))
    pnrm = ctx.enter_context(tc.tile_pool(name="nrm", bufs=2))
    pfeat = ctx.enter_context(tc.tile_pool(name="feat", bufs=3))
    psq = ctx.enter_context(tc.tile_pool(name="sq", bufs=2))
    pdscr = ctx.enter_context(tc.tile_pool(name="dscr", bufs=2))
    poht = ctx.enter_context(tc.tile_pool(name="oht", bufs=2))
    pG = ctx.enter_context(tc.tile_pool(name="G", bufs=1, space="PSUM"))
    pS = ctx.enter_context(tc.tile_pool(name="S", bufs=2, space="PSUM"))

    # small index tensors in one packed load on the scalar queue; the big
    # tables go out on the vector/gpsimd queues so nothing competes with
    # the feat stream on the sync queues
    smlt = const.tile([128, 200, 1], _BF16)
    nc.scalar.dma_start(smlt[:], sml[:, :, :])
    relt = smlt[:, 0:72, :]
    iot = smlt[:, 72:200, :].rearrange("p a b -> p b a")
    cntt = const.tile([128, 16], _F32)
    nc.scalar.dma_start(cntt[:], cnt[:, :])
    ct_all = const.tile([128, NB, D], _BF16)
    nc.gpsimd.dma_start(ct_all[:], cbf.rearrange("(p s) d -> p s d", p=128))
    nt_all = const.tile([128, NB, D], _BF16)
    nc.gpsimd.dma_start(nt_all[:], nbf.rearrange("(p s) d -> p s d", p=128))

    ones = const.tile([128, 1], _F32)
    nc.vector.memset(ones[:], 1.0)
    onesb = const.tile([128, 1], _BF16)
    nc.vector.memset(onesb[:], 1.0)
    # staging cols: 0-7 sum(f^2) per group; 8 counts.|c|^2; 9,15 ||G||^2
    # halves; 10-13 +cross per bank pair (host applies -2); 14 ||colsum||^2.
    staging = const.tile([128, 16], _F32)
    nc.vector.memset(staging[:], 0.0)

    # --- center-loss main loop; S PSUM tile covers two banks ---
    for g in range(NB):
        if g % 2 == 0:
            ohg = poht.tile([128, 2 * SUB, 128], _FP8, tag="ohg")
            nc.vector.tensor_tensor(
                out=ohg[:],
                in0=relt[:, ds(g * SUB, 2 * SUB), :].to_broadcast([128, 2 * SUB, 128]),
                in1=iot[:, :, :].to_broadcast([128, 2 * SUB, 128]),
                op=mybir.AluOpType.is_equal,
            )
            st = pS.tile([128, 2, D], _F32, tag="S")
        ft = pfeat.tile([128, SUB, D], _FP8, tag="ft")
        nc.sync.dma_start(
            ft[:], feat[ds(g * GROUP, GROUP), :].rearrange("(p s) d -> p s d", p=128)
        )
        sqs = psq.tile([128, SUB, D], _FP8, tag="sqs")
        nc.scalar.activation(
            sqs[:], ft[:], AF.Square, accum_out=staging[:, g : g + 1]
        )
        for s in range(SUB):
            nc.tensor.matmul(
                st[:, g % 2, :],
                ohg[:, (g % 2) * SUB + s, :],
                ft[:, s, :],
                start=(s == 0),
                stop=(s == SUB - 1),
            )
        if g % 2 == 1:
            dscr = pdscr.tile([CPB, 2, D], _F32, tag="dscr")
            nc.vector.tensor_tensor(
                out=dscr[:],
                in0=st[:CPB, :, :],
                in1=ct_all[:CPB, ds(g - 1, 2), :],
                op=mybir.AluOpType.mult,
            )
            nc.vector.tensor_reduce(
                out=staging[:CPB, 10 + g // 2 : 11 + g // 2],
                in_=dscr[:],
                axis=mybir.AxisListType.XY,
                op=mybir.AluOpType.add,
            )
        if g in (2, 4):
            # angular Gram (two 2-bank passes) slots into the Tensor
            # stream here: the normalized table is resident and the
            # scatter stream has slack
            kis = (0, 1) if g == 2 else (2, 3)
            Gt = pG.tile([128, 2, D], _F32, tag="G")
            for j in range(NB):
                for kx, ki in enumerate(kis):
                    nc.tensor.matmul(
                        Gt[:, kx, :],
                        nt_all[:CPB, j, ts(ki, 128)],
                        nt_all[:CPB, j, :],
                        start=(j == 0),
                        stop=(j == NB - 1),
                    )
            col = 9 if g == 2 else 15
            gsq = pnrm.tile([128, 2, D], _F32, tag="gsq")
            nc.scalar.activation(
                gsq[:], Gt[:], AF.Square, accum_out=staging[:, col : col + 1]
            )

    # counts . |c|^2 (norms^2 host-packed beside the counts)
    cscr = pnrm.tile([CPB, NB], _F32, tag="cscr")
    nc.vector.tensor_tensor(
        out=cscr[:],
        in0=cntt[:CPB, 0:NB],
        in1=cntt[:CPB, NB:16],
        op=mybir.AluOpType.mult,
    )
    nc.vector.tensor_reduce(
        out=staging[:CPB, 8:9],
        in_=cscr[:],
        axis=mybir.AxisListType.X,
        op=mybir.AluOpType.add,
    )

    # colsum of the normalized table (after ||G||^2 freed the Gram banks)
    csf = pG.tile([1, D], _F32, tag="cs")
    for j in range(NB):
        nc.tensor.matmul(
            csf[0:1, :],
            onesb[:CPB, :],
            nt_all[:CPB, j, :],
            start=(j == 0),
            stop=(j == NB - 1),
        )
    css = pnrm.tile([1, D], _F32, tag="css")
    nc.scalar.activation(
        css[0:1, :], csf[0:1, :], AF.Square, accum_out=staging[0:1, 14:15]
    )

    pf = pG.tile([1, 16], _F32, tag="cs")
    nc.tensor.matmul(pf[:], ones[:], staging[:], start=True, stop=True)
    osb = const.tile([1, 16], _F32)
    nc.vector.tensor_copy(osb[:], pf[:])
    nc.sync.dma_start(out[:, :], osb[:, :])


def build():
    if "nc" in _NC_CACHE:
        return _NC_CACHE["nc"]
    nc = bacc.Bacc(
        "TRN2",
        target_bir_lowering=False,
        debug=False,
        enable_asserts=False,
        num_devices=N_CORES,
    )
    feat = nc.dram_tensor("feat", [PR, D], _FP8, kind="ExternalInput").ap()
    cnt = nc.dram_tensor("cnt", [128, 16], _F32, kind="ExternalInput").ap()
    sml = nc.dram_tensor("sml", [128, 200, 1], _BF16, kind="ExternalInput").ap()
    cbf = nc.dram_tensor("ctab", [128 * NB, D], _BF16, kind="ExternalInput").ap()
    nbf = nc.dram_tensor("ntab", [128 * NB, D], _BF16, kind="ExternalInput").ap()
    out = nc.dram_tensor("out", [1, 16], _F32, kind="ExternalOutput").ap()
    with tile.TileContext(nc) as tc, ExitStack() as ctx:
        _build_body(ctx, tc, feat, cnt, sml, cbf, nbf, out)
    nc.compile()
    _NC_CACHE["nc"] = nc
    return nc


def _bank_assignment(y):
    """Greedy-balanced partition of the C classes into NB banks of CPB each."""
    counts = np.bincount(y, minlength=C)
    order = np.argsort(-counts, kind="stable")
    bank_tot = np.zeros(NB, dtype=np.int64)
    bank_n = np.zeros(NB, dtype=np.int64)
    bankclasses = np.zeros((NB, CPB), dtype=np.int64)
    cls_bank = np.zeros(C, dtype=np.int64)
    cls_pos = np.zeros(C, dtype=np.int64)
    for c in order:
        open_banks = np.flatnonzero(bank_n < CPB)
        j = open_banks[np.argmin(bank_tot[open_banks])]
        bankclasses[j, bank_n[j]] = c
        cls_bank[c] = j
        cls_pos[c] = bank_n[j]
        bank_n[j] += 1
        bank_tot[j] += counts[c]
    assert bank_tot.max() <= GROUP, f"bank overflow: {bank_tot.max()} > {GROUP}"
    return bankclasses, cls_bank, cls_pos, counts


def make_in_maps(y, feat, centers):
    feat = np.ascontiguousarray(feat, dtype=np.float32)
    centers = np.ascontiguousarray(centers, dtype=np.float32)
    y = np.asarray(y).astype(np.int64)
    norm2 = np.sum(centers.astype(np.float64) ** 2, axis=1, keepdims=True)
    ncenters = (centers / np.sqrt(norm2)).astype(ml_dtypes.bfloat16)
    in_maps = []
    for i in range(N_CORES):
        ys = y[i * BS : (i + 1) * BS]
        fs = feat[i * BS : (i + 1) * BS]
        bankclasses, cls_bank, cls_pos, counts = _bank_assignment(ys)

        # bank-major padded tables: dram row r (r%128 < 125) = class
        # bankclasses[r // 128][r % 128]
        ctab = np.zeros((128 * NB, D), dtype=ml_dtypes.bfloat16)
        ntab = np.zeros((128 * NB, D), dtype=ml_dtypes.bfloat16)
        rr = np.arange(128 * NB)
        vr = rr % 128 < CPB
        src = bankclasses[rr[vr] // 128, rr[vr] % 128]
        ctab[vr] = centers[src].astype(ml_dtypes.bfloat16)
        ntab[vr] = ncenters[src]

        # bucket rows by bank; group g rows sit at slots [g*GROUP, g*GROUP+n_g)
        row_bank = cls_bank[ys]
        grp_order = np.argsort(row_bank, kind="stable")
        n_per = np.bincount(row_bank, minlength=NB)
        starts = np.zeros(NB + 1, dtype=np.int64)
        starts[1:] = np.cumsum(n_per)
        slot = np.full(PR, -1, dtype=np.int64)  # slot -> source row
        for g in range(NB):
            rows = grp_order[starts[g] : starts[g + 1]]
            slot[g * GROUP : g * GROUP + len(rows)] = rows

        featp = np.zeros((PR, D), dtype=ml_dtypes.float8_e4m3)
        valid = slot >= 0
        featp[valid] = fs[slot[valid]].astype(ml_dtypes.float8_e4m3)

        # sml cols 0-71: rel (onehot column of the row at group g, part p,
        # slot s = padded row g*GROUP + 9p + s; -1 matches nothing);
        # cols 72-199: iota 0..127
        sml = np.zeros((128, 200, 1), dtype=ml_dtypes.bfloat16)
        sml[:, 0:72, 0] = -1.0
        k = np.flatnonzero(valid)
        g_k = k // GROUP
        r_k = k % GROUP
        p_k = r_k // SUB
        s_k = r_k % SUB
        sml[p_k, g_k * SUB + s_k, 0] = cls_pos[ys[slot[k]]]
        sml[:, 72:200, 0] = np.arange(128, dtype=np.float32)[None, :]

        cnt_pb = np.zeros((128, 16), dtype=np.float32)
        cnt_pb[cls_pos, cls_bank] = counts
        cnt_pb[cls_pos, NB + cls_bank] = norm2[:, 0]

        in_maps.append(
            {
                "feat": featp,
                "cnt": cnt_pb,
                "sml": sml,
                "ctab": ctab,
                "ntab": ntab,
            }
        )
    return in_maps


def combine(outs):
    """outs: list of 8 [1,16] f32 arrays -> scalar loss (np.float32)."""
    cen = 0.0
    for o in outs:
        o = np.asarray(o, dtype=np.float64)
        cen += o[0, 0:9].sum() - 2.0 * o[0, 10:14].sum()
    o0 = np.asarray(outs[0], dtype=np.float64)
    gsq, ssq = o0[0, 9] + o0[0, 15], o0[0, 14]
    ang = gsq - 2.0 * CT * ssq + C * C * CT * CT - C * (1.0 - CT) ** 2
    loss = 0.5 * cen / B + ang / (0.5 * C * (C - 1))
    return np.float32(loss)


def kernel(y, feat, centers):
    nc = build()
    in_maps = make_in_maps(y, feat, centers)
    res = run_bass_kernel_spmd(nc, in_maps, core_ids=list(range(N_CORES)))
    return combine([res.results[i]["out"] for i in range(N_CORES)])


# revision 26
# speedup vs baseline: 2.7439x; 1.0818x over previous
"""Trainium2 Bass kernel for AngelLoss (center loss + angular loss).

loss = 0.5*sum((feat - centers[y])^2)/B
     + sum_offdiag((c_i.c_j/(|c_i||c_j|) - ct)^2) / (0.5*C*(C-1))

Sharding (8 NeuronCores, data-parallel over batch):
  - center term, gather-free:  sum||f||^2 - 2*sum_c c_c.S_c + sum_c n_c||c_c||^2
    where S_c = sum of feat rows with label c.  Host buckets each core's
    8192 rows into 8 class-banks (125 classes each, greedy-balanced), pads
    each bank's rows to 9x128 and ships them fp8 along with fp8 onehots.
    S for two banks accumulates in one 2-bank PSUM tile via 2x9
    onehot^T @ feat fp8 matmuls and drains with one DVE multiply +
    free-axis reduce against the resident bf16 centers.  sum||f||^2
    splits 7/2 across ScalarE (square+accum) and DVE (mult+reduce).
  - angular term via the Frobenius identity (N = row-normalized centers):
      sum_ij (sim-ct)^2 = ||N^T N||_F^2 - 2ct ||sum_i N_i||^2 + C^2 ct^2
    computed redundantly on every core from the resident fp8 normalized
    table with 2x16 accumulating matmuls into 2-bank PSUM Gram halves.
  - per-core [1,32] partial sums are combined on the host.
"""

from contextlib import ExitStack

import ml_dtypes
import numpy as np

import concourse.bass as bass
import concourse.tile as tile
from concourse import bacc, mybir
from concourse.bass import ds, ts
from concourse.bass_utils import run_bass_kernel_spmd

N_CORES = 8
B, C, D = 65536, 1000, 512
BS = B // N_CORES  # 8192 rows per core
NB = 8  # class banks
CPB = C // NB  # 125 classes per bank
SUB = 9  # 128-row subtiles per bank group (1152 slots >= ~1024+slack rows)
GROUP = 128 * SUB  # 1152
PR = NB * GROUP  # 9216 padded rows per core
NSC = 7  # feat slots squared on ScalarE; the rest (SUB-NSC) go to DVE

# ct = 2*radius(C-1)^2 - 1 from the reference, evaluated in f64, cast f32.
CT = float(np.float32(-0.0010010010010047532))

_F32 = mybir.dt.float32
_BF16 = mybir.dt.bfloat16
_FP8 = mybir.dt.float8e4

_NC_CACHE = {}


def _build_body(ctx, tc, feat, cnt, oh, cbf, nbf, out):
    nc = tc.nc
    AF = mybir.ActivationFunctionType

    const = ctx.enter_context(tc.tile_pool(name="const", bufs=1))
    pnrm = ctx.enter_context(tc.tile_pool(name="nrm", bufs=2))
    pfeat = ctx.enter_context(tc.tile_pool(name="feat", bufs=3))
    psq = ctx.enter_context(tc.tile_pool(name="sq", bufs=2))
    pdscr = ctx.enter_context(tc.tile_pool(name="dscr", bufs=2))
    pG = ctx.enter_context(tc.tile_pool(name="G", bufs=1, space="PSUM"))
    pS = ctx.enter_context(tc.tile_pool(name="S", bufs=2, space="PSUM"))

    # scalar queue: onehots + counts; gpsimd queue: the two tables;
    # sync queue: the feat stream (nothing else competes with it)
    oht = const.tile([128, NB * SUB, 128], _FP8)
    nc.scalar.dma_start(oht[:], oh[:, :, :])
    cntt = const.tile([128, 16], _F32)
    nc.scalar.dma_start(cntt[:], cnt[:, :])
    ct_all = const.tile([128, NB, D], _BF16)
    nc.gpsimd.dma_start(ct_all[:], cbf.rearrange("(p s) d -> p s d", p=128))
    nt_all = const.tile([128, NB, D], _FP8)
    nc.gpsimd.dma_start(nt_all[:], nbf.rearrange("(p s) d -> p s d", p=128))

    ones = const.tile([128, 1], _F32)
    nc.vector.memset(ones[:], 1.0)
    onesp = const.tile([128, 1], _FP8)
    nc.vector.memset(onesp[:], 1.0)
    # staging cols: 0-7 scalar sum(f^2); 8 counts.|c|^2; 9,15 ||G||^2
    # halves; 10-13 +cross per bank pair (host applies -2); 14 ||colsum||^2;
    # 16-23 vector sum(f^2).
    staging = const.tile([128, 32], _F32)
    nc.vector.memset(staging[:], 0.0)

    # --- center-loss main loop; S PSUM tile covers two banks ---
    for g in range(NB):
        if g % 2 == 0:
            st = pS.tile([128, 2, D], _F32, tag="S")
        ft = pfeat.tile([128, SUB, D], _FP8, tag="ft")
        nc.sync.dma_start(
            ft[:], feat[ds(g * GROUP, GROUP), :].rearrange("(p s) d -> p s d", p=128)
        )
        sqs = psq.tile([128, NSC, D], _FP8, tag="sqs")
        nc.scalar.activation(
            sqs[:], ft[:, :NSC, :], AF.Square, accum_out=staging[:, g : g + 1]
        )
        sqv = psq.tile([128, SUB - NSC, D], _FP8, tag="sqv")
        nc.vector.tensor_tensor(
            out=sqv[:],
            in0=ft[:, NSC:, :],
            in1=ft[:, NSC:, :],
            op=mybir.AluOpType.mult,
        )
        nc.vector.tensor_reduce(
            out=staging[:, 16 + g : 17 + g],
            in_=sqv[:],
            axis=mybir.AxisListType.XY,
            op=mybir.AluOpType.add,
        )
        for s in range(SUB):
            nc.tensor.matmul(
                st[:, g % 2, :],
                oht[:, g * SUB + s, :],
                ft[:, s, :],
                start=(s == 0),
                stop=(s == SUB - 1),
            )
        if g % 2 == 1:
            dscr = pdscr.tile([CPB, 2, D], _F32, tag="dscr")
            nc.vector.tensor_tensor(
                out=dscr[:],
                in0=st[:CPB, :, :],
                in1=ct_all[:CPB, ds(g - 1, 2), :],
                op=mybir.AluOpType.mult,
            )
            nc.vector.tensor_reduce(
                out=staging[:CPB, 10 + g // 2 : 11 + g // 2],
                in_=dscr[:],
                axis=mybir.AxisListType.XY,
                op=mybir.AluOpType.add,
            )
        if g in (2, 4):
            # angular Gram (two 2-bank passes) slots into the Tensor
            # stream here: the normalized table is resident and the
            # scatter stream has slack
            kis = (0, 1) if g == 2 else (2, 3)
            Gt = pG.tile([128, 2, D], _F32, tag="G")
            for j in range(NB):
                for kx, ki in enumerate(kis):
                    nc.tensor.matmul(
                        Gt[:, kx, :],
                        nt_all[:CPB, j, ts(ki, 128)],
                        nt_all[:CPB, j, :],
                        start=(j == 0),
                        stop=(j == NB - 1),
                    )
            col = 9 if g == 2 else 15
            gsq = pnrm.tile([128, 2, D], _F32, tag="gsq")
            nc.scalar.activation(
                gsq[:], Gt[:], AF.Square, accum_out=staging[:, col : col + 1]
            )
        if g == 3:
            # colsum of the normalized table on the idle cs PSUM bank
            csf = pG.tile([1, D], _F32, tag="cs")
            for j in range(NB):
                nc.tensor.matmul(
                    csf[0:1, :],
                    onesp[:CPB, :],
                    nt_all[:CPB, j, :],
                    start=(j == 0),
                    stop=(j == NB - 1),
                )
            css = pnrm.tile([1, D], _F32, tag="css")
            nc.scalar.activation(
                css[0:1, :], csf[0:1, :], AF.Square, accum_out=staging[0:1, 14:15]
            )
        if g == 4:
            # counts . |c|^2 (norms^2 host-packed beside the counts)
            cscr = pnrm.tile([CPB, NB], _F32, tag="cscr")
            nc.vector.tensor_tensor(
                out=cscr[:],
                in0=cntt[:CPB, 0:NB],
                in1=cntt[:CPB, NB:16],
                op=mybir.AluOpType.mult,
            )
            nc.vector.tensor_reduce(
                out=staging[:CPB, 8:9],
                in_=cscr[:],
                axis=mybir.AxisListType.X,
                op=mybir.AluOpType.add,
            )

    pf = pG.tile([1, 32], _F32, tag="cs")
    nc.tensor.matmul(pf[:], ones[:], staging[:], start=True, stop=True)
    osb = const.tile([1, 32], _F32)
    nc.vector.tensor_copy(osb[:], pf[:])
    nc.sync.dma_start(out[:, :], osb[:, :])


def build():
    if "nc" in _NC_CACHE:
        return _NC_CACHE["nc"]
    nc = bacc.Bacc(
        "TRN2",
        target_bir_lowering=False,
        debug=False,
        enable_asserts=False,
        num_devices=N_CORES,
    )
    feat = nc.dram_tensor("feat", [PR, D], _FP8, kind="ExternalInput").ap()
    cnt = nc.dram_tensor("cnt", [128, 16], _F32, kind="ExternalInput").ap()
    oh = nc.dram_tensor("oh", [128, NB * SUB, 128], _FP8, kind="ExternalInput").ap()
    cbf = nc.dram_tensor("ctab", [128 * NB, D], _BF16, kind="ExternalInput").ap()
    nbf = nc.dram_tensor("ntab", [128 * NB, D], _FP8, kind="ExternalInput").ap()
    out = nc.dram_tensor("out", [1, 32], _F32, kind="ExternalOutput").ap()
    with tile.TileContext(nc) as tc, ExitStack() as ctx:
        _build_body(ctx, tc, feat, cnt, oh, cbf, nbf, out)
    nc.compile()
    _NC_CACHE["nc"] = nc
    return nc


def _bank_assignment(y):
    """Greedy-balanced partition of the C classes into NB banks of CPB each."""
    counts = np.bincount(y, minlength=C)
    order = np.argsort(-counts, kind="stable")
    bank_tot = np.zeros(NB, dtype=np.int64)
    bank_n = np.zeros(NB, dtype=np.int64)
    bankclasses = np.zeros((NB, CPB), dtype=np.int64)
    cls_bank = np.zeros(C, dtype=np.int64)
    cls_pos = np.zeros(C, dtype=np.int64)
    for c in order:
        open_banks = np.flatnonzero(bank_n < CPB)
        j = open_banks[np.argmin(bank_tot[open_banks])]
        bankclasses[j, bank_n[j]] = c
        cls_bank[c] = j
        cls_pos[c] = bank_n[j]
        bank_n[j] += 1
        bank_tot[j] += counts[c]
    assert bank_tot.max() <= GROUP, f"bank overflow: {bank_tot.max()} > {GROUP}"
    return bankclasses, cls_bank, cls_pos, counts


def make_in_maps(y, feat, centers):
    feat = np.ascontiguousarray(feat, dtype=np.float32)
    centers = np.ascontiguousarray(centers, dtype=np.float32)
    y = np.asarray(y).astype(np.int64)
    norm2 = np.sum(centers.astype(np.float64) ** 2, axis=1, keepdims=True)
    ncenters = (centers / np.sqrt(norm2)).astype(ml_dtypes.float8_e4m3)
    in_maps = []
    for i in range(N_CORES):
        ys = y[i * BS : (i + 1) * BS]
        fs = feat[i * BS : (i + 1) * BS]
        bankclasses, cls_bank, cls_pos, counts = _bank_assignment(ys)

        # bank-major padded tables: dram row r (r%128 < 125) = class
        # bankclasses[r // 128][r % 128]
        ctab = np.zeros((128 * NB, D), dtype=ml_dtypes.bfloat16)
        ntab = np.zeros((128 * NB, D), dtype=ml_dtypes.float8_e4m3)
        rr = np.arange(128 * NB)
        vr = rr % 128 < CPB
        src = bankclasses[rr[vr] // 128, rr[vr] % 128]
        ctab[vr] = centers[src].astype(ml_dtypes.bfloat16)
        ntab[vr] = ncenters[src]

        # bucket rows by bank; group g rows sit at slots [g*GROUP, g*GROUP+n_g)
        row_bank = cls_bank[ys]
        grp_order = np.argsort(row_bank, kind="stable")
        n_per = np.bincount(row_bank, minlength=NB)
        starts = np.zeros(NB + 1, dtype=np.int64)
        starts[1:] = np.cumsum(n_per)
        slot = np.full(PR, -1, dtype=np.int64)  # slot -> source row
        for g in range(NB):
            rows = grp_order[starts[g] : starts[g + 1]]
            slot[g * GROUP : g * GROUP + len(rows)] = rows

        featp = np.zeros((PR, D), dtype=ml_dtypes.float8_e4m3)
        valid = slot >= 0
        featp[valid] = fs[slot[valid]].astype(ml_dtypes.float8_e4m3)

        # onehot for matmul (g, s): row at (part p, slot s) is padded row
        # g*GROUP + 9p + s; pads get no column
        oh = np.zeros((128, NB * SUB, 128), dtype=ml_dtypes.float8_e4m3)
        k = np.flatnonzero(valid)
        g_k = k // GROUP
        r_k = k % GROUP
        p_k = r_k // SUB
        s_k = r_k % SUB
        oh[p_k, g_k * SUB + s_k, cls_pos[ys[slot[k]]]] = 1.0

        cnt_pb = np.zeros((128, 16), dtype=np.float32)
        cnt_pb[cls_pos, cls_bank] = counts
        cnt_pb[cls_pos, NB + cls_bank] = norm2[:, 0]

        in_maps.append(
            {
                "feat": featp,
                "cnt": cnt_pb,
                "oh": oh,
                "ctab": ctab,
                "ntab": ntab,
            }
        )
    return in_maps


def combine(outs):
    """outs: list of 8 [1,32] f32 arrays -> scalar loss (np.float32)."""
    cen = 0.0
    for o in outs:
        o = np.asarray(o, dtype=np.float64)
        cen += o[0, 0:9].sum() + o[0, 16:24].sum() - 2.0 * o[0, 10:14].sum()
    o0 = np.asarray(outs[0], dtype=np.float64)
    gsq, ssq = o0[0, 9] + o0[0, 15], o0[0, 14]
    ang = gsq - 2.0 * CT * ssq + C * C * CT * CT - C * (1.0 - CT) ** 2
    loss = 0.5 * cen / B + ang / (0.5 * C * (C - 1))
    return np.float32(loss)


def kernel(y, feat, centers):
    nc = build()
    in_maps = make_in_maps(y, feat, centers)
    res = run_bass_kernel_spmd(nc, in_maps, core_ids=list(range(N_CORES)))
    return combine([res.results[i]["out"] for i in range(N_CORES)])


# revision 34
# speedup vs baseline: 2.7916x; 1.0174x over previous
"""Trainium2 Bass kernel for AngelLoss (center loss + angular loss).

loss = 0.5*sum((feat - centers[y])^2)/B
     + sum_offdiag((c_i.c_j/(|c_i||c_j|) - ct)^2) / (0.5*C*(C-1))

Sharding (8 NeuronCores, data-parallel over batch):
  - center term, gather-free:  sum||f||^2 - 2*sum_c c_c.S_c + sum_c n_c||c_c||^2
    where S_c = sum of feat rows with label c.  Host buckets each core's
    8192 rows into 8 class-banks (125 classes each, greedy-balanced), pads
    each bank's rows to 9x128 and ships them fp8 along with fp8 onehots.
    S for two banks accumulates in one 2-bank PSUM tile via 2x9
    onehot^T @ feat fp8 matmuls and drains with one DVE multiply +
    free-axis reduce against the resident bf16 centers.  sum||f||^2
    splits 7/2 across ScalarE (square+accum) and DVE (mult+reduce).
  - angular term via the Frobenius identity (N = row-normalized centers):
      sum_ij (sim-ct)^2 = ||N^T N||_F^2 - 2ct ||sum_i N_i||^2 + C^2 ct^2
    computed redundantly on every core from the resident fp8 normalized
    table with 2x16 accumulating matmuls into 2-bank PSUM Gram halves.
  - per-core [1,32] partial sums are combined on the host.
"""

from contextlib import ExitStack

import ml_dtypes
import numpy as np

import concourse.bass as bass
import concourse.tile as tile
from concourse import bacc, mybir
from concourse.bass import ds, ts
from concourse.bass_utils import run_bass_kernel_spmd

N_CORES = 8
B, C, D = 65536, 1000, 512
BS = B // N_CORES  # 8192 rows per core
NB = 8  # class banks
CPB = C // NB  # 125 classes per bank
SUB = 9  # 128-row subtiles per bank group (1152 slots >= ~1024+slack rows)
GROUP = 128 * SUB  # 1152
PR = NB * GROUP  # 9216 padded rows per core
NSC = 7  # feat slots squared on ScalarE; the rest (SUB-NSC) go to DVE

# ct = 2*radius(C-1)^2 - 1 from the reference, evaluated in f64, cast f32.
CT = float(np.float32(-0.0010010010010047532))

_F32 = mybir.dt.float32
_BF16 = mybir.dt.bfloat16
_FP8 = mybir.dt.float8e4

_NC_CACHE = {}


def _build_body(ctx, tc, feat, cnt, oh, cbf, nbf, out):
    nc = tc.nc
    AF = mybir.ActivationFunctionType

    const = ctx.enter_context(tc.tile_pool(name="const", bufs=1))
    pnrm = ctx.enter_context(tc.tile_pool(name="nrm", bufs=2))
    pfeat = ctx.enter_context(tc.tile_pool(name="feat", bufs=3))
    psq = ctx.enter_context(tc.tile_pool(name="sq", bufs=2))
    pdscr = ctx.enter_context(tc.tile_pool(name="dscr", bufs=2))
    pG = ctx.enter_context(tc.tile_pool(name="G", bufs=1, space="PSUM"))
    pS = ctx.enter_context(tc.tile_pool(name="S", bufs=2, space="PSUM"))

    # scalar queue: onehots + counts; gpsimd queue: the two tables;
    # sync queue: the feat stream (nothing else competes with it)
    oht = const.tile([128, NB * SUB, 128], _FP8)
    for h in range(4):
        nc.scalar.dma_start(
            oht[:, ds(h * 2 * SUB, 2 * SUB), :], oh[:, ds(h * 2 * SUB, 2 * SUB), :]
        )
    cntt = const.tile([128, 16], _F32)
    nc.scalar.dma_start(cntt[:], cnt[:, :])
    ct_all = const.tile([128, NB, D], _FP8)
    nc.gpsimd.dma_start(ct_all[:], cbf.rearrange("(p s) d -> p s d", p=128))
    nt_all = const.tile([128, NB, D], _FP8)
    nc.gpsimd.dma_start(nt_all[:], nbf.rearrange("(p s) d -> p s d", p=128))

    ones = const.tile([128, 1], _F32)
    nc.vector.memset(ones[:], 1.0)
    onesp2 = const.tile([128, 2, 1], _FP8)
    nc.vector.memset(onesp2[:], 1.0)
    # staging cols: 0-7 scalar sum(f^2); 8 counts.|c|^2; 9,15 ||G||^2
    # halves; 10-13 +cross per bank pair (host applies -2); 14 ||colsum||^2;
    # 16-23 vector sum(f^2).
    staging = const.tile([128, 32], _F32)
    nc.vector.memset(staging[:], 0.0)

    # --- center-loss main loop; S PSUM tile covers two banks ---
    for g in range(NB):
        if g % 2 == 0:
            st = pS.tile([128, 2, D], _F32, tag="S")
        ft = pfeat.tile([128, SUB, D], _FP8, tag="ft")
        nc.sync.dma_start(
            ft[:], feat[ds(g * GROUP, GROUP), :].rearrange("(p s) d -> p s d", p=128)
        )
        sqs = psq.tile([128, NSC, D], _FP8, tag="sqs")
        nc.scalar.activation(
            sqs[:], ft[:, :NSC, :], AF.Square, accum_out=staging[:, g : g + 1]
        )
        sqv = psq.tile([128, SUB - NSC, D], _FP8, tag="sqv")
        nc.vector.tensor_tensor(
            out=sqv[:],
            in0=ft[:, NSC:, :],
            in1=ft[:, NSC:, :],
            op=mybir.AluOpType.mult,
        )
        nc.vector.tensor_reduce(
            out=staging[:, 16 + g : 17 + g],
            in_=sqv[:],
            axis=mybir.AxisListType.XY,
            op=mybir.AluOpType.add,
        )
        for sp in range(0, SUB - 1, 2):
            nc.tensor.matmul(
                st[:, g % 2, :],
                oht[:, ds(g * SUB + sp, 2), :],
                ft[:, ds(sp, 2), :],
                start=(sp == 0),
                stop=False,
                perf_mode=mybir.MatmulPerfMode.DoubleRow,
            )
        nc.tensor.matmul(
            st[:, g % 2, :],
            oht[:, g * SUB + SUB - 1, :],
            ft[:, SUB - 1, :],
            start=False,
            stop=True,
        )
        if g % 2 == 1:
            dscr = pdscr.tile([CPB, 2, D], _F32, tag="dscr")
            nc.vector.tensor_tensor(
                out=dscr[:],
                in0=st[:CPB, :, :],
                in1=ct_all[:CPB, ds(g - 1, 2), :],
                op=mybir.AluOpType.mult,
            )
            nc.vector.tensor_reduce(
                out=staging[:CPB, 10 + g // 2 : 11 + g // 2],
                in_=dscr[:],
                axis=mybir.AxisListType.XY,
                op=mybir.AluOpType.add,
            )
        if g in (2, 4):
            # angular Gram (two 2-bank passes) slots into the Tensor
            # stream here: the normalized table is resident and the
            # scatter stream has slack
            kis = (0, 1) if g == 2 else (2, 3)
            Gt = pG.tile([128, 2, D], _F32, tag="G")
            for jp in range(0, NB, 2):
                for kx, ki in enumerate(kis):
                    nc.tensor.matmul(
                        Gt[:, kx, :],
                        nt_all[:CPB, ds(jp, 2), ts(ki, 128)],
                        nt_all[:CPB, ds(jp, 2), :],
                        start=(jp == 0),
                        stop=(jp == NB - 2),
                        perf_mode=mybir.MatmulPerfMode.DoubleRow,
                    )
            col = 9 if g == 2 else 15
            gsq = pnrm.tile([128, 2, D], _F32, tag="gsq")
            nc.scalar.activation(
                gsq[:], Gt[:], AF.Square, accum_out=staging[:, col : col + 1]
            )
        if g == 3:
            # colsum of the normalized table on the idle cs PSUM bank
            csf = pG.tile([1, D], _F32, tag="cs")
            for j in range(NB):
                nc.tensor.matmul(
                    csf[0:1, :],
                    onesp2[:CPB, 0, :],
                    nt_all[:CPB, j, :],
                    start=(j == 0),
                    stop=(j == NB - 1),
                )
            css = pnrm.tile([1, D], _F32, tag="css")
            nc.scalar.activation(
                css[0:1, :], csf[0:1, :], AF.Square, accum_out=staging[0:1, 14:15]
            )
        if g == 4:
            # counts . |c|^2 (norms^2 host-packed beside the counts)
            cscr = pnrm.tile([CPB, NB], _F32, tag="cscr")
            nc.vector.tensor_tensor(
                out=cscr[:],
                in0=cntt[:CPB, 0:NB],
                in1=cntt[:CPB, NB:16],
                op=mybir.AluOpType.mult,
            )
            nc.vector.tensor_reduce(
                out=staging[:CPB, 8:9],
                in_=cscr[:],
                axis=mybir.AxisListType.X,
                op=mybir.AluOpType.add,
            )

    pf = pG.tile([1, 32], _F32, tag="cs")
    nc.tensor.matmul(pf[:], ones[:], staging[:], start=True, stop=True)
    osb = const.tile([1, 32], _F32)
    nc.vector.tensor_copy(osb[:], pf[:])
    nc.sync.dma_start(out[:, :], osb[:, :])


def build():
    if "nc" in _NC_CACHE:
        return _NC_CACHE["nc"]
    nc = bacc.Bacc(
        "TRN2",
        target_bir_lowering=False,
        debug=False,
        enable_asserts=False,
        num_devices=N_CORES,
    )
    feat = nc.dram_tensor("feat", [PR, D], _FP8, kind="ExternalInput").ap()
    cnt = nc.dram_tensor("cnt", [128, 16], _F32, kind="ExternalInput").ap()
    oh = nc.dram_tensor("oh", [128, NB * SUB, 128], _FP8, kind="ExternalInput").ap()
    cbf = nc.dram_tensor("ctab", [128 * NB, D], _FP8, kind="ExternalInput").ap()
    nbf = nc.dram_tensor("ntab", [128 * NB, D], _FP8, kind="ExternalInput").ap()
    out = nc.dram_tensor("out", [1, 32], _F32, kind="ExternalOutput").ap()
    with tile.TileContext(nc) as tc, ExitStack() as ctx:
        _build_body(ctx, tc, feat, cnt, oh, cbf, nbf, out)
    nc.compile()
    _NC_CACHE["nc"] = nc
    return nc


def _bank_assignment(y):
    """Greedy-balanced partition of the C classes into NB banks of CPB each."""
    counts = np.bincount(y, minlength=C)
    order = np.argsort(-counts, kind="stable")
    bank_tot = np.zeros(NB, dtype=np.int64)
    bank_n = np.zeros(NB, dtype=np.int64)
    bankclasses = np.zeros((NB, CPB), dtype=np.int64)
    cls_bank = np.zeros(C, dtype=np.int64)
    cls_pos = np.zeros(C, dtype=np.int64)
    for c in order:
        open_banks = np.flatnonzero(bank_n < CPB)
        j = open_banks[np.argmin(bank_tot[open_banks])]
        bankclasses[j, bank_n[j]] = c
        cls_bank[c] = j
        cls_pos[c] = bank_n[j]
        bank_n[j] += 1
        bank_tot[j] += counts[c]
    assert bank_tot.max() <= GROUP, f"bank overflow: {bank_tot.max()} > {GROUP}"
    return bankclasses, cls_bank, cls_pos, counts


def make_in_maps(y, feat, centers):
    feat = np.ascontiguousarray(feat, dtype=np.float32)
    centers = np.ascontiguousarray(centers, dtype=np.float32)
    y = np.asarray(y).astype(np.int64)
    norm2 = np.sum(centers.astype(np.float64) ** 2, axis=1, keepdims=True)
    ncenters = (centers / np.sqrt(norm2)).astype(ml_dtypes.float8_e4m3)
    in_maps = []
    for i in range(N_CORES):
        ys = y[i * BS : (i + 1) * BS]
        fs = feat[i * BS : (i + 1) * BS]
        bankclasses, cls_bank, cls_pos, counts = _bank_assignment(ys)

        # bank-major padded tables: dram row r (r%128 < 125) = class
        # bankclasses[r // 128][r % 128]
        ctab = np.zeros((128 * NB, D), dtype=ml_dtypes.float8_e4m3)
        ntab = np.zeros((128 * NB, D), dtype=ml_dtypes.float8_e4m3)
        rr = np.arange(128 * NB)
        vr = rr % 128 < CPB
        src = bankclasses[rr[vr] // 128, rr[vr] % 128]
        ctab[vr] = centers[src].astype(ml_dtypes.float8_e4m3)
        ntab[vr] = ncenters[src]

        # bucket rows by bank; group g rows sit at slots [g*GROUP, g*GROUP+n_g)
        row_bank = cls_bank[ys]
        grp_order = np.argsort(row_bank, kind="stable")
        n_per = np.bincount(row_bank, minlength=NB)
        starts = np.zeros(NB + 1, dtype=np.int64)
        starts[1:] = np.cumsum(n_per)
        slot = np.full(PR, -1, dtype=np.int64)  # slot -> source row
        for g in range(NB):
            rows = grp_order[starts[g] : starts[g + 1]]
            slot[g * GROUP : g * GROUP + len(rows)] = rows

        featp = np.zeros((PR, D), dtype=ml_dtypes.float8_e4m3)
        valid = slot >= 0
        featp[valid] = fs[slot[valid]].astype(ml_dtypes.float8_e4m3)

        # onehot for matmul (g, s): row at (part p, slot s) is padded row
        # g*GROUP + 9p + s; pads get no column
        oh = np.zeros((128, NB * SUB, 128), dtype=ml_dtypes.float8_e4m3)
        k = np.flatnonzero(valid)
        g_k = k // GROUP
        r_k = k % GROUP
        p_k = r_k // SUB
        s_k = r_k % SUB
        oh[p_k, g_k * SUB + s_k, cls_pos[ys[slot[k]]]] = 1.0

        cnt_pb = np.zeros((128, 16), dtype=np.float32)
        cnt_pb[cls_pos, cls_bank] = counts
        cnt_pb[cls_pos, NB + cls_bank] = norm2[:, 0]

        in_maps.append(
            {
                "feat": featp,
                "cnt": cnt_pb,
                "oh": oh,
                "ctab": ctab,
                "ntab": ntab,
            }
        )
    return in_maps


def combine(outs):
    """outs: list of 8 [1,32] f32 arrays -> scalar loss (np.float32)."""
    cen = 0.0
    for o in outs:
        o = np.asarray(o, dtype=np.float64)
        cen += o[0, 0:9].sum() + o[0, 16:24].sum() - 2.0 * o[0, 10:14].sum()
    o0 = np.asarray(outs[0], dtype=np.float64)
    gsq, ssq = o0[0, 9] + o0[0, 15], o0[0, 14]
    ang = gsq - 2.0 * CT * ssq + C * C * CT * CT - C * (1.0 - CT) ** 2
    loss = 0.5 * cen / B + ang / (0.5 * C * (C - 1))
    return np.float32(loss)


def kernel(y, feat, centers):
    nc = build()
    in_maps = make_in_maps(y, feat, centers)
    res = run_bass_kernel_spmd(nc, in_maps, core_ids=list(range(N_CORES)))
    return combine([res.results[i]["out"] for i in range(N_CORES)])
